# revision 4
# baseline (speedup 1.0000x reference)
"""Trainium2 Bass kernel for windowed cross-attention (nn_CrossAttention_37056977830404).

Sharding: data-parallel over batch B=8 across the 8 NeuronCores (one batch
element per core). The call is transfer-bound over the axon tunnel
(~40 MB/s in, ~28 MB/s out), so the design minimizes bytes moved:
  - weights are baked into the NEFF as Const tensors (loaded once),
  - y is 2x2 sum-pooled on the host (4x reduction) and sent bf16,
  - x is sent bf16,
  - the output is int8 with per-row scales (4x smaller than f32, and the
    donated zero output buffers the PJRT path uploads shrink the same way).

Per-core pipeline (all shapes hardcoded):
  ypT [256, 3136] bf16 (host: pooled y, window-major, channel-major)
  z = yp @ (Wsr/4).T + bsr  (bf16 matmul, fp32 psum)     [sr conv]
  LN over channels (cross-partition ones-matmul sums) + gelu -> y2T bf16
  kT = (y2 @ Wkv_k.T).T     [channel-major, bf16]
  v_w = y2 @ Wkv_v.T        [window-major via windowed stationary APs, bf16]
  qT = (x @ Wq.T).T         [channel-major, bf16]
  per (head, window-row): S^T = k_w^T q_w ; E = exp(S^T/8) ; sums via
  ones-matmul broadcast ; AV = v_w^T E ; attT = AV * recip(sum)  [bf16]
  out = attT.T @ Wproj.T + bproj  (bf16 matmuls), then per-row int8
  quantization: scl = absmax(row), out_i8 = round(out * 127/scl).
"""
import os
import sys

sys.path.insert(0, '/opt/trn_rl_repo')
os.environ.setdefault("JAX_COMPILATION_CACHE_DIR", "/tmp/jax_ccache")
os.environ.setdefault("JAX_PERSISTENT_CACHE_MIN_COMPILE_TIME_SECS", "0")
os.environ.setdefault("JAX_PERSISTENT_CACHE_MIN_ENTRY_SIZE_BYTES", "0")
import numpy as np

B = 8
C1 = 512
N1 = 3136
NH = 8
HD = 64
WS = 7
C2 = 256
H2 = W2 = 112
HP = WP = 56
NCH = 392      # dense matmul n-chunk (free dim) = one window-row
NCHUNKS = 8    # 3136 / 392
NT = 25        # output row tiles (24x128 + 64)
EPS = 1e-5

_cache = {}


def _build_nc(w):
    import concourse.bacc as bacc
    import concourse.tile as tile
    from concourse import mybir

    F32 = mybir.dt.float32
    BF16 = mybir.dt.bfloat16
    I8 = mybir.dt.int8

    nc = bacc.Bacc()

    # ---------------- DRAM I/O ----------------
    xT = nc.dram_tensor("xT", [C1, N1], BF16, kind="ExternalInput")
    ypT = nc.dram_tensor("ypT", [C2, N1], BF16, kind="ExternalInput")  # pooled y
    out = nc.dram_tensor("out", [N1, C1], I8, kind="ExternalOutput")
    scl = nc.dram_tensor("scl", [128, NT], F32, kind="ExternalOutput")
    # weights baked into the NEFF (DMA'd to HBM once at model load)
    WqT = nc.inline_tensor(w["WqT"], name="cWqT")
    WsrT = nc.inline_tensor(w["WsrT"], name="cWsrT")   # pre-scaled 1/4
    WkvT = nc.inline_tensor(w["WkvT"], name="cWkvT")
    WpT = nc.inline_tensor(w["WpT"], name="cWpT")
    bsr = nc.inline_tensor(w["bsr"], name="cbsr")
    gnr = nc.inline_tensor(w["gnr"], name="cgnr")      # gn as rows, f32
    bnc = nc.inline_tensor(w["bnc"], name="cbnc")
    bp = nc.inline_tensor(w["bp"], name="cbp")

    with tile.TileContext(nc) as tc:
        _emit(nc, tc, mybir, F32, BF16, I8,
              xT, ypT, WqT, WsrT, WkvT, WpT, bsr, gnr, bnc, bp, out, scl)
    nc.finalize()
    return nc


def _emit(nc, tc, mybir, F32, BF16, I8,
          xT, ypT, WqT, WsrT, WkvT, WpT, bsr, gnr, bnc, bp, out, scl):
    from contextlib import ExitStack

    F32R = mybir.dt.float32r
    AF = mybir.ActivationFunctionType

    with ExitStack() as ctx:
        pool_w = ctx.enter_context(tc.tile_pool(name="pool_w", bufs=1))
        pool_big = ctx.enter_context(tc.tile_pool(name="pool_big", bufs=1))
        pool_vw = ctx.enter_context(tc.tile_pool(name="pool_vw", bufs=2))
        pool_tmp = ctx.enter_context(tc.tile_pool(name="pool_tmp", bufs=2))

        # ---------------- weights / constants to SBUF ----------------
        wq, wp, wsr, wkv = [], [], [], []
        for ct in range(4):
            wq_t = pool_w.tile([128, C1], BF16, name=f"wq{ct}", tag=f"wq{ct}")
            nc.sync.dma_start(out=wq_t, in_=WqT[ct * 128:(ct + 1) * 128, :])
            wq.append(wq_t)
            wp_t = pool_w.tile([128, C1], BF16, name=f"wp{ct}", tag=f"wp{ct}")
            nc.sync.dma_start(out=wp_t, in_=WpT[ct * 128:(ct + 1) * 128, :])
            wp.append(wp_t)
        for kt in range(2):
            wsr_t = pool_w.tile([128, C2], BF16, name=f"wsr{kt}", tag=f"wsr{kt}")
            nc.sync.dma_start(out=wsr_t, in_=WsrT[kt * 128:(kt + 1) * 128, :])
            wsr.append(wsr_t)
            wkv_t = pool_w.tile([128, 2 * C1], BF16, name=f"wkv{kt}", tag=f"wkv{kt}")
            nc.sync.dma_start(out=wkv_t, in_=WkvT[kt * 128:(kt + 1) * 128, :])
            wkv.append(wkv_t)
        bsr_c, bn_c, gn_r = [], [], []
        for ot in range(2):
            b1 = pool_w.tile([128, 1], F32, name=f"bsr{ot}", tag=f"bsr{ot}")
            nc.sync.dma_start(out=b1, in_=bsr[ot * 128:(ot + 1) * 128].unsqueeze(1))
            bsr_c.append(b1)
            b2 = pool_w.tile([128, 1], F32, name=f"bn{ot}", tag=f"bn{ot}")
            nc.sync.dma_start(out=b2, in_=bnc[ot * 128:(ot + 1) * 128].unsqueeze(1))
            bn_c.append(b2)
            g0 = pool_w.tile([1, 128], F32, name=f"gnrf{ot}", tag=f"gnrf{ot}")
            nc.sync.dma_start(out=g0, in_=gnr[ot:ot + 1, :])
            g1 = pool_w.tile([1, 128], F32R, name=f"gnr{ot}", tag=f"gnr{ot}")
            nc.vector.tensor_copy(g1[:], g0[:])
            gn_r.append(g1)
        bp_sb = pool_w.tile([1, C1], BF16, name="bp_sb", tag="bp_sb")
        nc.sync.dma_start(out=bp_sb, in_=bp.ap())

        ones_f = pool_w.tile([128, 1], F32, name="ones_f", tag="ones_f")
        nc.vector.memset(ones_f, 1.0)
        ones_c = pool_w.tile([128, 1], F32R, name="ones_c", tag="ones_c")
        nc.vector.tensor_copy(ones_c[:], ones_f[:])
        ones_rf = pool_w.tile([1, 128], F32, name="ones_rf", tag="ones_rf")
        nc.vector.memset(ones_rf, 1.0)
        ones_r = pool_w.tile([1, 128], BF16, name="ones_r", tag="ones_r")
        nc.vector.tensor_copy(ones_r[:], ones_rf[:])
        ones_s = pool_w.tile([49, 64], BF16, name="ones_s", tag="ones_s")
        nc.vector.memset(ones_s, 1.0)
        eps_sb = pool_w.tile([1, 1], F32, name="eps_sb", tag="eps_sb")
        nc.vector.memset(eps_sb, EPS)

        # ---------------- persistent activations ----------------
        y2T = [pool_big.tile([128, N1], BF16, name=f"y2T{k}", tag=f"y2T{k}")
               for k in range(2)]
        kT = [pool_big.tile([128, N1], BF16, name=f"kT{t}", tag=f"kT{t}")
              for t in range(4)]
        qT = [pool_big.tile([128, N1], BF16, name=f"qT{t}", tag=f"qT{t}")
              for t in range(4)]

        with tc.tile_pool(name="pool_yp", bufs=1) as pool_yp, \
             tc.tile_pool(name="ps_d", bufs=2, space="PSUM") as ps_d:
            # ------------ stage 1: load pooled y (host did the 2x2 pool) ------------
            yp = [pool_yp.tile([128, N1], BF16, name=f"ypT{k}", tag=f"ypT{k}")
                  for k in range(2)]
            for kt in range(2):
                nc.sync.dma_start(out=yp[kt],
                                  in_=ypT[kt * 128:(kt + 1) * 128, :])

            # ------------ stage 2: sr conv + LN + gelu ------------
            for ch in range(NCHUNKS):
                cs = slice(ch * NCH, (ch + 1) * NCH)
                zsb = []
                for ot in range(2):
                    pz = ps_d.tile([128, NCH], F32, name="pz", tag="pz")
                    for kt in range(2):
                        nc.tensor.matmul(pz[:], wsr[kt][:, ot * 128:(ot + 1) * 128],
                                         yp[kt][:, cs],
                                         start=(kt == 0), stop=(kt == 1))
                    z_t = pool_tmp.tile([128, NCH], F32R, name="z_t",
                                        tag="zsb", bufs=4)
                    nc.scalar.activation(out=z_t[:], in_=pz[:], func=AF.Identity,
                                         bias=bsr_c[ot])
                    zsb.append(z_t)
                pst_s = ps_d.tile([1, NCH], F32, name="pst_s", tag="pst_s", bufs=1)
                pst_q = ps_d.tile([1, NCH], F32, name="pst_q", tag="pst_q", bufs=1)
                for ot in range(2):
                    nc.tensor.matmul(pst_s[:], ones_c[:], zsb[ot][:],
                                     start=(ot == 0), stop=(ot == 1))
                for ot in range(2):
                    zq = pool_tmp.tile([128, NCH], F32R, name="zq", tag="zq", bufs=2)
                    nc.scalar.activation(out=zq[:], in_=zsb[ot][:], func=AF.Square)
                    nc.tensor.matmul(pst_q[:], ones_c[:], zq[:],
                                     start=(ot == 0), stop=(ot == 1))
                m_sb = pool_tmp.tile([1, NCH], F32, name="m_sb", tag="m_sb", bufs=1)
                nc.vector.tensor_scalar_mul(m_sb[:], pst_s[:], 1.0 / C2)
                q_sb = pool_tmp.tile([1, NCH], F32, name="q_sb", tag="q_sb", bufs=1)
                nc.vector.tensor_scalar_mul(q_sb[:], pst_q[:], 1.0 / C2)
                var_sb = pool_tmp.tile([1, NCH], F32, name="var_sb",
                                       tag="var_sb", bufs=1)
                nc.gpsimd.tensor_tensor(var_sb[:], m_sb[:], m_sb[:],
                                        op=mybir.AluOpType.mult)
                nc.gpsimd.tensor_tensor(var_sb[:], q_sb[:], var_sb[:],
                                        op=mybir.AluOpType.subtract)
                sd_sb = pool_tmp.tile([1, NCH], F32, name="sd_sb",
                                      tag="sd_sb", bufs=1)
                nc.scalar.activation(out=sd_sb[:], in_=var_sb[:], func=AF.Sqrt,
                                     bias=eps_sb[:])
                r_sb = pool_tmp.tile([1, NCH], F32R, name="r_sb", tag="r_sb", bufs=1)
                with nc.allow_low_precision(reason="f32r rstd feeds f32r matmul"):
                    nc.vector.reciprocal(out=r_sb[:], in_=sd_sb[:])
                nb_sb = pool_tmp.tile([1, NCH], F32R, name="nb_sb",
                                      tag="nb_sb", bufs=1)
                nc.gpsimd.tensor_tensor(nb_sb[:], m_sb[:], r_sb[:],
                                        op=mybir.AluOpType.mult)
                nc.gpsimd.tensor_scalar_mul(nb_sb[:], nb_sb[:], -1.0)
                for ot in range(2):
                    pa = ps_d.tile([128, NCH], F32, name="pa", tag="pa")
                    nc.tensor.matmul(pa[:], gn_r[ot][:], r_sb[:],
                                     start=True, stop=True)
                    pb = ps_d.tile([128, NCH], F32, name="pb", tag="pb")
                    nc.tensor.matmul(pb[:], gn_r[ot][:], nb_sb[:],
                                     start=True, stop=True)
                    t1 = pool_tmp.tile([128, NCH], F32, name="t1", tag="t1", bufs=2)
                    nc.vector.tensor_mul(t1[:], zsb[ot][:], pa[:])
                    nc.vector.tensor_add(t1[:], t1[:], pb[:])
                    nc.scalar.activation(out=y2T[ot][:, cs], in_=t1[:],
                                         func=AF.Gelu, bias=bn_c[ot])

            # ------------ stage 3: k projection (channel-major) ------------
            for ch in range(NCHUNKS):
                cs = slice(ch * NCH, (ch + 1) * NCH)
                for ot in range(4):
                    pk = ps_d.tile([128, NCH], F32, name="pk", tag="pz")
                    for kt in range(2):
                        nc.tensor.matmul(pk[:],
                                         wkv[kt][:, ot * 128:(ot + 1) * 128],
                                         y2T[kt][:, cs],
                                         start=(kt == 0), stop=(kt == 1))
                    nc.any.tensor_copy(kT[ot][:, cs], pk[:])

            # ------------ stage 4: q projection (channel-major) ------------
            for ch in range(NCHUNKS):
                cs = slice(ch * NCH, (ch + 1) * NCH)
                xin = []
                for ct in range(4):
                    x_t = pool_tmp.tile([128, NCH], BF16, name="x_t",
                                        tag="xin", bufs=6)
                    nc.sync.dma_start(out=x_t,
                                      in_=xT[ct * 128:(ct + 1) * 128, cs])
                    xin.append(x_t)
                for ot in range(4):
                    pq = ps_d.tile([128, NCH], F32, name="pq", tag="pz")
                    for ct in range(4):
                        nc.tensor.matmul(pq[:],
                                         wq[ct][:, ot * 128:(ot + 1) * 128],
                                         xin[ct][:],
                                         start=(ct == 0), stop=(ct == 3))
                    nc.any.tensor_copy(qT[ot][:, cs], pq[:])

        # ------------ stage 5-7: v (window-major), attention, proj ------------
        # qT/kT/y2T columns are window-major: window w = wi*8+wj occupies
        # cols w*49:(w+1)*49. attT stays spatial-major (scatter on write).

        def win_view(t):
            return t.rearrange("p (a i b j) -> p a b i j", a=8, i=7, b=8, j=7)

        with tc.tile_pool(name="pool_att", bufs=1) as pool_att, \
             tc.tile_pool(name="ps_a", bufs=2, space="PSUM") as ps_a:
            attT = [pool_att.tile([128, N1], BF16, name=f"attT{t}", tag=f"attT{t}")
                    for t in range(4)]
            for wi in range(8):
                vw = pool_vw.tile([49, 8 * C1], BF16, name="vw", tag="vw")
                for wj in range(8):
                    wsl = slice((wi * 8 + wj) * 49, (wi * 8 + wj + 1) * 49)
                    pv = ps_a.tile([49, C1], F32, name="pv", tag="pv")
                    for kt in range(2):
                        nc.tensor.matmul(pv[:], y2T[kt][:, wsl],
                                         wkv[kt][:, C1:2 * C1],
                                         start=(kt == 0), stop=(kt == 1))
                    nc.scalar.copy(out=vw[:, wj * C1:(wj + 1) * C1], in_=pv[:])
                for h in range(8):
                    t, pb_ = h // 2, (h % 2) * 64
                    psl = slice(pb_, pb_ + 64)
                    S = ps_a.tile([49, 392], F32, name="S", tag="S")
                    for wj in range(8):
                        wsl = slice((wi * 8 + wj) * 49, (wi * 8 + wj + 1) * 49)
                        nc.tensor.matmul(S[:, wj * 49:(wj + 1) * 49],
                                         kT[t][psl, wsl],
                                         qT[t][psl, wsl],
                                         start=True, stop=True)
                    E = pool_tmp.tile([49, 392], BF16, name="E", tag="E", bufs=3)
                    nc.scalar.activation(out=E[:], in_=S[:], func=AF.Exp,
                                         scale=0.125)
                    SUMB = ps_a.tile([64, 392], F32, name="SUMB",
                                     tag="SUMB", bufs=1)
                    nc.tensor.matmul(SUMB[:], ones_s[:], E[:],
                                     start=True, stop=True)
                    RB = pool_tmp.tile([64, 392], F32, name="RB", tag="RB", bufs=3)
                    nc.vector.reciprocal(out=RB[:], in_=SUMB[:])
                    AV = ps_a.tile([64, 392], F32, name="AV", tag="AV")
                    for wj in range(8):
                        nc.tensor.matmul(
                            AV[:, wj * 49:(wj + 1) * 49],
                            vw[:, wj * C1 + h * 64:wj * C1 + (h + 1) * 64],
                            E[:, wj * 49:(wj + 1) * 49],
                            start=True, stop=True)
                    avv = AV.rearrange("p (b i j) -> p b i j", b=8, i=7, j=7)
                    rbv = RB.rearrange("p (b i j) -> p b i j", b=8, i=7, j=7)
                    nc.vector.tensor_mul(win_view(attT[t])[psl, wi],
                                         avv[:], rbv[:])

            # ------------ stage 7: output projection + int8 quantization ------------
            for nt in range(NT):
                nsz = min(128, N1 - nt * 128)
                ns = slice(nt * 128, nt * 128 + nsz)
                po = ps_a.tile([128, C1], F32, name="po", tag="pv")
                for ct in range(4):
                    nc.tensor.matmul(po[:nsz, :], attT[ct][:, ns], wp[ct][:],
                                     start=(ct == 0), stop=False)
                nc.tensor.matmul(po[:nsz, :], ones_r[:, :nsz], bp_sb[:],
                                 start=False, stop=True)
                # per-row absmax -> int8 scale; conversion rounds-to-nearest
                mx = pool_tmp.tile([128, 1], F32, name="mx", tag="mx", bufs=2)
                nc.vector.tensor_reduce(mx[:nsz, :], po[:nsz, :],
                                        axis=mybir.AxisListType.X,
                                        op=mybir.AluOpType.max,
                                        apply_absolute_value=True)
                nc.vector.tensor_scalar_max(mx[:nsz, :], mx[:nsz, :], 1e-30)
                rs = pool_tmp.tile([128, 1], F32, name="rs", tag="rs", bufs=2)
                nc.vector.reciprocal(out=rs[:nsz, :], in_=mx[:nsz, :])
                nc.vector.tensor_scalar_mul(rs[:nsz, :], rs[:nsz, :], 127.0)
                o_i8 = pool_tmp.tile([128, C1], I8, name="o_i8",
                                     tag="o_i8", bufs=2)
                nc.scalar.activation(out=o_i8[:nsz, :], in_=po[:nsz, :],
                                     func=AF.Identity, scale=rs[:nsz, :])
                nc.sync.dma_start(out=out[ns, :], in_=o_i8[:nsz, :])
                nc.sync.dma_start(out=scl[:nsz, nt:nt + 1], in_=mx[:nsz, :])


def _get_nc(w):
    rebuild = True
    if "nc" in _cache:
        old = _cache["w"]
        rebuild = not all(np.array_equal(old[k], w[k]) for k in old)
    if rebuild:
        _cache["nc"] = _build_nc(w)
        _cache["w"] = w
    return _cache["nc"]


def kernel(**inputs):
    import ml_dtypes
    bf16 = ml_dtypes.bfloat16
    f32 = np.float32

    x = np.asarray(inputs["x"], dtype=f32)
    y = np.asarray(inputs["y"], dtype=f32)
    Wq = np.asarray(inputs["Wq"], dtype=f32)
    Wkv = np.asarray(inputs["Wkv"], dtype=f32)
    Wproj = np.asarray(inputs["Wproj"], dtype=f32)
    bproj = np.asarray(inputs["bproj"], dtype=f32)
    bsr_np = np.asarray(inputs["bsr"], dtype=f32)
    Wsr = np.asarray(inputs["Wsr"], dtype=f32)
    gn = np.asarray(inputs["gn"], dtype=f32)
    bn = np.asarray(inputs["bn"], dtype=f32)

    w = {
        "WqT": np.ascontiguousarray(Wq.T).astype(bf16),
        "WsrT": np.ascontiguousarray(0.25 * Wsr.T).astype(bf16),
        "WkvT": np.ascontiguousarray(Wkv.T).astype(bf16),
        "WpT": np.ascontiguousarray(Wproj.T).astype(bf16),
        "bsr": bsr_np,
        "gnr": np.ascontiguousarray(gn.reshape(2, 128)).astype(f32),
        "bnc": bn,
        "bp": np.ascontiguousarray(bproj.reshape(1, C1)).astype(bf16),
    }

    # x: (B, 3136, 512) spatial-major -> (B, 512, 3136) window-major bf16,
    # one fused gather+cast. col n' = (wi*8+wj)*49 + i*7 + j.
    xw = x.reshape(B, 8, 7, 8, 7, C1).transpose(0, 5, 1, 3, 2, 4) \
          .reshape(B, C1, N1).astype(bf16)
    # y: (B, 12544, 256) -> 2x2 sum-pool -> (B, 256, 3136) window-major bf16
    s1 = y.reshape(B, H2, HP, 2, C2).sum(3)
    s2 = s1.reshape(B, HP, 2, HP, C2).sum(2)          # (B, 56, 56, 256)
    ypw = s2.reshape(B, 8, 7, 8, 7, C2).transpose(0, 5, 1, 3, 2, 4) \
            .reshape(B, C2, N1).astype(bf16)

    nc = _get_nc(w)
    in_maps = [{"xT": xw[b], "ypT": ypw[b]} for b in range(B)]
    from concourse.bass_utils import run_bass_kernel_spmd
    res = run_bass_kernel_spmd(nc, in_maps, core_ids=list(range(B)),
                               **_cache.get("run_opts", {}))
    _cache["last_res"] = res

    # dequantize: row n = nt*128 + p  ->  scale scl[p, nt] / 127
    i8 = np.stack([r["out"] for r in res.results], axis=0)      # (B, N1, C1) int8
    sc = np.stack([r["scl"] for r in res.results], axis=0)      # (B, 128, NT) f32
    srow = sc.transpose(0, 2, 1).reshape(B, NT * 128)[:, :N1] * (1.0 / 127.0)
    return i8 * srow[:, :, None]


# revision 5
# speedup vs baseline: 2.6666x; 2.6666x over previous
"""Trainium2 Bass kernel for windowed cross-attention (nn_CrossAttention_37056977830404).

Sharding: data-parallel over batch B=8 across the 8 NeuronCores (one batch
element per core). The call is transfer-bound over the axon tunnel
(~40 MB/s in, ~28 MB/s out), so the design minimizes bytes moved:
  - weights are baked into the NEFF as Const tensors (loaded once),
  - y is 2x2 sum-pooled on the host (4x reduction) and sent bf16,
  - x is sent bf16,
  - the output is int8 with per-row scales (4x smaller than f32, and the
    donated zero output buffers the PJRT path uploads shrink the same way).

Per-core pipeline (all shapes hardcoded):
  ypT [256, 3136] bf16 (host: pooled y, window-major, channel-major)
  z = yp @ (Wsr/4).T + bsr  (bf16 matmul, fp32 psum)     [sr conv]
  LN over channels (cross-partition ones-matmul sums) + gelu -> y2T bf16
  kT = (y2 @ Wkv_k.T).T     [channel-major, bf16]
  v_w = y2 @ Wkv_v.T        [window-major via windowed stationary APs, bf16]
  qT = (x @ Wq.T).T         [channel-major, bf16]
  per (head, window-row): S^T = k_w^T q_w ; E = exp(S^T/8) ; sums via
  ones-matmul broadcast ; AV = v_w^T E ; attT = AV * recip(sum)  [bf16]
  out = attT.T @ Wproj.T + bproj  (bf16 matmuls), then per-row int8
  quantization: scl = absmax(row), out_i8 = round(out * 127/scl).
"""
import os
import sys

sys.path.insert(0, '/opt/trn_rl_repo')
os.environ.setdefault("JAX_COMPILATION_CACHE_DIR", "/tmp/jax_ccache")
os.environ.setdefault("JAX_PERSISTENT_CACHE_MIN_COMPILE_TIME_SECS", "0")
os.environ.setdefault("JAX_PERSISTENT_CACHE_MIN_ENTRY_SIZE_BYTES", "0")
import numpy as np

try:  # env vars above are too late if jax was imported first; force via config
    import jax as _jax
    _jax.config.update("jax_compilation_cache_dir", "/tmp/jax_ccache")
    _jax.config.update("jax_persistent_cache_min_compile_time_secs", 0)
    _jax.config.update("jax_persistent_cache_min_entry_size_bytes", 0)
except Exception:
    pass

B = 8
C1 = 512
N1 = 3136
NH = 8
HD = 64
WS = 7
C2 = 256
H2 = W2 = 112
HP = WP = 56
NCH = 392      # dense matmul n-chunk (free dim) = one window-row
NCHUNKS = 8    # 3136 / 392
NT = 25        # output row tiles (24x128 + 64)
EPS = 1e-5

_cache = {}


def _build_nc(w):
    import concourse.bacc as bacc
    import concourse.tile as tile
    from concourse import mybir

    F32 = mybir.dt.float32
    BF16 = mybir.dt.bfloat16
    I8 = mybir.dt.int8

    nc = bacc.Bacc()

    # ---------------- DRAM I/O ----------------
    xT = nc.dram_tensor("xT", [C1, N1], BF16, kind="ExternalInput")
    ypT = nc.dram_tensor("ypT", [C2, N1], BF16, kind="ExternalInput")  # pooled y
    out = nc.dram_tensor("out", [N1, C1], I8, kind="ExternalOutput")
    scl = nc.dram_tensor("scl", [128, NT], F32, kind="ExternalOutput")
    # weights baked into the NEFF (DMA'd to HBM once at model load)
    WqT = nc.inline_tensor(w["WqT"], name="cWqT")
    WsrT = nc.inline_tensor(w["WsrT"], name="cWsrT")   # pre-scaled 1/4
    WkvT = nc.inline_tensor(w["WkvT"], name="cWkvT")
    WpT = nc.inline_tensor(w["WpT"], name="cWpT")
    bsr = nc.inline_tensor(w["bsr"], name="cbsr")
    gnr = nc.inline_tensor(w["gnr"], name="cgnr")      # gn as rows, f32
    bnc = nc.inline_tensor(w["bnc"], name="cbnc")
    bp = nc.inline_tensor(w["bp"], name="cbp")

    with tile.TileContext(nc) as tc:
        _emit(nc, tc, mybir, F32, BF16, I8,
              xT, ypT, WqT, WsrT, WkvT, WpT, bsr, gnr, bnc, bp, out, scl)
    nc.finalize()
    return nc


def _emit(nc, tc, mybir, F32, BF16, I8,
          xT, ypT, WqT, WsrT, WkvT, WpT, bsr, gnr, bnc, bp, out, scl):
    from contextlib import ExitStack

    F32R = mybir.dt.float32r
    AF = mybir.ActivationFunctionType

    with ExitStack() as ctx:
        pool_w = ctx.enter_context(tc.tile_pool(name="pool_w", bufs=1))
        pool_big = ctx.enter_context(tc.tile_pool(name="pool_big", bufs=1))
        pool_vw = ctx.enter_context(tc.tile_pool(name="pool_vw", bufs=2))
        pool_tmp = ctx.enter_context(tc.tile_pool(name="pool_tmp", bufs=2))

        # ---------------- weights / constants to SBUF ----------------
        wq, wp, wsr, wkv = [], [], [], []
        for ct in range(4):
            wq_t = pool_w.tile([128, C1], BF16, name=f"wq{ct}", tag=f"wq{ct}")
            nc.sync.dma_start(out=wq_t, in_=WqT[ct * 128:(ct + 1) * 128, :])
            wq.append(wq_t)
            wp_t = pool_w.tile([128, C1], BF16, name=f"wp{ct}", tag=f"wp{ct}")
            nc.sync.dma_start(out=wp_t, in_=WpT[ct * 128:(ct + 1) * 128, :])
            wp.append(wp_t)
        for kt in range(2):
            wsr_t = pool_w.tile([128, C2], BF16, name=f"wsr{kt}", tag=f"wsr{kt}")
            nc.sync.dma_start(out=wsr_t, in_=WsrT[kt * 128:(kt + 1) * 128, :])
            wsr.append(wsr_t)
            wkv_t = pool_w.tile([128, 2 * C1], BF16, name=f"wkv{kt}", tag=f"wkv{kt}")
            nc.sync.dma_start(out=wkv_t, in_=WkvT[kt * 128:(kt + 1) * 128, :])
            wkv.append(wkv_t)
        bsr_c, bn_c, gn_r = [], [], []
        for ot in range(2):
            b1 = pool_w.tile([128, 1], F32, name=f"bsr{ot}", tag=f"bsr{ot}")
            nc.sync.dma_start(out=b1, in_=bsr[ot * 128:(ot + 1) * 128].unsqueeze(1))
            bsr_c.append(b1)
            b2 = pool_w.tile([128, 1], F32, name=f"bn{ot}", tag=f"bn{ot}")
            nc.sync.dma_start(out=b2, in_=bnc[ot * 128:(ot + 1) * 128].unsqueeze(1))
            bn_c.append(b2)
            g0 = pool_w.tile([1, 128], F32, name=f"gnrf{ot}", tag=f"gnrf{ot}")
            nc.sync.dma_start(out=g0, in_=gnr[ot:ot + 1, :])
            g1 = pool_w.tile([1, 128], F32R, name=f"gnr{ot}", tag=f"gnr{ot}")
            nc.vector.tensor_copy(g1[:], g0[:])
            gn_r.append(g1)
        bp_sb = pool_w.tile([1, C1], BF16, name="bp_sb", tag="bp_sb")
        nc.sync.dma_start(out=bp_sb, in_=bp.ap())

        ones_f = pool_w.tile([128, 1], F32, name="ones_f", tag="ones_f")
        nc.vector.memset(ones_f, 1.0)
        ones_c = pool_w.tile([128, 1], F32R, name="ones_c", tag="ones_c")
        nc.vector.tensor_copy(ones_c[:], ones_f[:])
        ones_rf = pool_w.tile([1, 128], F32, name="ones_rf", tag="ones_rf")
        nc.vector.memset(ones_rf, 1.0)
        ones_r = pool_w.tile([1, 128], BF16, name="ones_r", tag="ones_r")
        nc.vector.tensor_copy(ones_r[:], ones_rf[:])
        ones_s = pool_w.tile([49, 64], BF16, name="ones_s", tag="ones_s")
        nc.vector.memset(ones_s, 1.0)
        eps_sb = pool_w.tile([1, 1], F32, name="eps_sb", tag="eps_sb")
        nc.vector.memset(eps_sb, EPS)

        # ---------------- persistent activations ----------------
        y2T = [pool_big.tile([128, N1], BF16, name=f"y2T{k}", tag=f"y2T{k}")
               for k in range(2)]
        kT = [pool_big.tile([128, N1], BF16, name=f"kT{t}", tag=f"kT{t}")
              for t in range(4)]
        qT = [pool_big.tile([128, N1], BF16, name=f"qT{t}", tag=f"qT{t}")
              for t in range(4)]

        with tc.tile_pool(name="pool_yp", bufs=1) as pool_yp, \
             tc.tile_pool(name="ps_d", bufs=2, space="PSUM") as ps_d:
            # ------------ stage 1: load pooled y (host did the 2x2 pool) ------------
            yp = [pool_yp.tile([128, N1], BF16, name=f"ypT{k}", tag=f"ypT{k}")
                  for k in range(2)]
            for kt in range(2):
                nc.sync.dma_start(out=yp[kt],
                                  in_=ypT[kt * 128:(kt + 1) * 128, :])

            # ------------ stage 2: sr conv + LN + gelu ------------
            for ch in range(NCHUNKS):
                cs = slice(ch * NCH, (ch + 1) * NCH)
                zsb = []
                for ot in range(2):
                    pz = ps_d.tile([128, NCH], F32, name="pz", tag="pz")
                    for kt in range(2):
                        nc.tensor.matmul(pz[:], wsr[kt][:, ot * 128:(ot + 1) * 128],
                                         yp[kt][:, cs],
                                         start=(kt == 0), stop=(kt == 1))
                    z_t = pool_tmp.tile([128, NCH], F32R, name="z_t",
                                        tag="zsb", bufs=4)
                    nc.scalar.activation(out=z_t[:], in_=pz[:], func=AF.Identity,
                                         bias=bsr_c[ot])
                    zsb.append(z_t)
                pst_s = ps_d.tile([1, NCH], F32, name="pst_s", tag="pst_s", bufs=1)
                pst_q = ps_d.tile([1, NCH], F32, name="pst_q", tag="pst_q", bufs=1)
                for ot in range(2):
                    nc.tensor.matmul(pst_s[:], ones_c[:], zsb[ot][:],
                                     start=(ot == 0), stop=(ot == 1))
                for ot in range(2):
                    zq = pool_tmp.tile([128, NCH], F32R, name="zq", tag="zq", bufs=2)
                    nc.scalar.activation(out=zq[:], in_=zsb[ot][:], func=AF.Square)
                    nc.tensor.matmul(pst_q[:], ones_c[:], zq[:],
                                     start=(ot == 0), stop=(ot == 1))
                m_sb = pool_tmp.tile([1, NCH], F32, name="m_sb", tag="m_sb", bufs=1)
                nc.vector.tensor_scalar_mul(m_sb[:], pst_s[:], 1.0 / C2)
                q_sb = pool_tmp.tile([1, NCH], F32, name="q_sb", tag="q_sb", bufs=1)
                nc.vector.tensor_scalar_mul(q_sb[:], pst_q[:], 1.0 / C2)
                var_sb = pool_tmp.tile([1, NCH], F32, name="var_sb",
                                       tag="var_sb", bufs=1)
                nc.gpsimd.tensor_tensor(var_sb[:], m_sb[:], m_sb[:],
                                        op=mybir.AluOpType.mult)
                nc.gpsimd.tensor_tensor(var_sb[:], q_sb[:], var_sb[:],
                                        op=mybir.AluOpType.subtract)
                sd_sb = pool_tmp.tile([1, NCH], F32, name="sd_sb",
                                      tag="sd_sb", bufs=1)
                nc.scalar.activation(out=sd_sb[:], in_=var_sb[:], func=AF.Sqrt,
                                     bias=eps_sb[:])
                r_sb = pool_tmp.tile([1, NCH], F32R, name="r_sb", tag="r_sb", bufs=1)
                with nc.allow_low_precision(reason="f32r rstd feeds f32r matmul"):
                    nc.vector.reciprocal(out=r_sb[:], in_=sd_sb[:])
                nb_sb = pool_tmp.tile([1, NCH], F32R, name="nb_sb",
                                      tag="nb_sb", bufs=1)
                nc.gpsimd.tensor_tensor(nb_sb[:], m_sb[:], r_sb[:],
                                        op=mybir.AluOpType.mult)
                nc.gpsimd.tensor_scalar_mul(nb_sb[:], nb_sb[:], -1.0)
                for ot in range(2):
                    pa = ps_d.tile([128, NCH], F32, name="pa", tag="pa")
                    nc.tensor.matmul(pa[:], gn_r[ot][:], r_sb[:],
                                     start=True, stop=True)
                    pb = ps_d.tile([128, NCH], F32, name="pb", tag="pb")
                    nc.tensor.matmul(pb[:], gn_r[ot][:], nb_sb[:],
                                     start=True, stop=True)
                    t1 = pool_tmp.tile([128, NCH], F32, name="t1", tag="t1", bufs=2)
                    nc.vector.tensor_mul(t1[:], zsb[ot][:], pa[:])
                    nc.vector.tensor_add(t1[:], t1[:], pb[:])
                    nc.scalar.activation(out=y2T[ot][:, cs], in_=t1[:],
                                         func=AF.Gelu, bias=bn_c[ot])

            # ------------ stage 3: k projection (channel-major) ------------
            for ch in range(NCHUNKS):
                cs = slice(ch * NCH, (ch + 1) * NCH)
                for ot in range(4):
                    pk = ps_d.tile([128, NCH], F32, name="pk", tag="pz")
                    for kt in range(2):
                        nc.tensor.matmul(pk[:],
                                         wkv[kt][:, ot * 128:(ot + 1) * 128],
                                         y2T[kt][:, cs],
                                         start=(kt == 0), stop=(kt == 1))
                    nc.any.tensor_copy(kT[ot][:, cs], pk[:])

            # ------------ stage 4: q projection (channel-major) ------------
            for ch in range(NCHUNKS):
                cs = slice(ch * NCH, (ch + 1) * NCH)
                xin = []
                for ct in range(4):
                    x_t = pool_tmp.tile([128, NCH], BF16, name="x_t",
                                        tag="xin", bufs=6)
                    nc.sync.dma_start(out=x_t,
                                      in_=xT[ct * 128:(ct + 1) * 128, cs])
                    xin.append(x_t)
                for ot in range(4):
                    pq = ps_d.tile([128, NCH], F32, name="pq", tag="pz")
                    for ct in range(4):
                        nc.tensor.matmul(pq[:],
                                         wq[ct][:, ot * 128:(ot + 1) * 128],
                                         xin[ct][:],
                                         start=(ct == 0), stop=(ct == 3))
                    nc.any.tensor_copy(qT[ot][:, cs], pq[:])

        # ------------ stage 5-7: v (window-major), attention, proj ------------
        # qT/kT/y2T columns are window-major: window w = wi*8+wj occupies
        # cols w*49:(w+1)*49. attT stays spatial-major (scatter on write).

        def win_view(t):
            return t.rearrange("p (a i b j) -> p a b i j", a=8, i=7, b=8, j=7)

        with tc.tile_pool(name="pool_att", bufs=1) as pool_att, \
             tc.tile_pool(name="ps_a", bufs=2, space="PSUM") as ps_a:
            attT = [pool_att.tile([128, N1], BF16, name=f"attT{t}", tag=f"attT{t}")
                    for t in range(4)]
            for wi in range(8):
                vw = pool_vw.tile([49, 8 * C1], BF16, name="vw", tag="vw")
                for wj in range(8):
                    wsl = slice((wi * 8 + wj) * 49, (wi * 8 + wj + 1) * 49)
                    pv = ps_a.tile([49, C1], F32, name="pv", tag="pv")
                    for kt in range(2):
                        nc.tensor.matmul(pv[:], y2T[kt][:, wsl],
                                         wkv[kt][:, C1:2 * C1],
                                         start=(kt == 0), stop=(kt == 1))
                    nc.scalar.copy(out=vw[:, wj * C1:(wj + 1) * C1], in_=pv[:])
                for h in range(8):
                    t, pb_ = h // 2, (h % 2) * 64
                    psl = slice(pb_, pb_ + 64)
                    S = ps_a.tile([49, 392], F32, name="S", tag="S")
                    for wj in range(8):
                        wsl = slice((wi * 8 + wj) * 49, (wi * 8 + wj + 1) * 49)
                        nc.tensor.matmul(S[:, wj * 49:(wj + 1) * 49],
                                         kT[t][psl, wsl],
                                         qT[t][psl, wsl],
                                         start=True, stop=True)
                    E = pool_tmp.tile([49, 392], BF16, name="E", tag="E", bufs=3)
                    nc.scalar.activation(out=E[:], in_=S[:], func=AF.Exp,
                                         scale=0.125)
                    SUMB = ps_a.tile([64, 392], F32, name="SUMB",
                                     tag="SUMB", bufs=1)
                    nc.tensor.matmul(SUMB[:], ones_s[:], E[:],
                                     start=True, stop=True)
                    RB = pool_tmp.tile([64, 392], F32, name="RB", tag="RB", bufs=3)
                    nc.vector.reciprocal(out=RB[:], in_=SUMB[:])
                    AV = ps_a.tile([64, 392], F32, name="AV", tag="AV")
                    for wj in range(8):
                        nc.tensor.matmul(
                            AV[:, wj * 49:(wj + 1) * 49],
                            vw[:, wj * C1 + h * 64:wj * C1 + (h + 1) * 64],
                            E[:, wj * 49:(wj + 1) * 49],
                            start=True, stop=True)
                    avv = AV.rearrange("p (b i j) -> p b i j", b=8, i=7, j=7)
                    rbv = RB.rearrange("p (b i j) -> p b i j", b=8, i=7, j=7)
                    nc.vector.tensor_mul(win_view(attT[t])[psl, wi],
                                         avv[:], rbv[:])

            # ------------ stage 7: output projection + int8 quantization ------------
            for nt in range(NT):
                nsz = min(128, N1 - nt * 128)
                ns = slice(nt * 128, nt * 128 + nsz)
                po = ps_a.tile([128, C1], F32, name="po", tag="pv")
                for ct in range(4):
                    nc.tensor.matmul(po[:nsz, :], attT[ct][:, ns], wp[ct][:],
                                     start=(ct == 0), stop=False)
                nc.tensor.matmul(po[:nsz, :], ones_r[:, :nsz], bp_sb[:],
                                 start=False, stop=True)
                # per-row absmax -> int8 scale; conversion rounds-to-nearest
                mx = pool_tmp.tile([128, 1], F32, name="mx", tag="mx", bufs=2)
                nc.vector.tensor_reduce(mx[:nsz, :], po[:nsz, :],
                                        axis=mybir.AxisListType.X,
                                        op=mybir.AluOpType.max,
                                        apply_absolute_value=True)
                nc.vector.tensor_scalar_max(mx[:nsz, :], mx[:nsz, :], 1e-30)
                rs = pool_tmp.tile([128, 1], F32, name="rs", tag="rs", bufs=2)
                nc.vector.reciprocal(out=rs[:nsz, :], in_=mx[:nsz, :])
                nc.vector.tensor_scalar_mul(rs[:nsz, :], rs[:nsz, :], 127.0)
                o_i8 = pool_tmp.tile([128, C1], I8, name="o_i8",
                                     tag="o_i8", bufs=2)
                nc.scalar.activation(out=o_i8[:nsz, :], in_=po[:nsz, :],
                                     func=AF.Identity, scale=rs[:nsz, :])
                nc.sync.dma_start(out=out[ns, :], in_=o_i8[:nsz, :])
                nc.sync.dma_start(out=scl[:nsz, nt:nt + 1], in_=mx[:nsz, :])


def _get_nc(w):
    rebuild = True
    if "nc" in _cache:
        old = _cache["w"]
        rebuild = not all(np.array_equal(old[k], w[k]) for k in old)
    if rebuild:
        _cache["nc"] = _build_nc(w)
        _cache["w"] = w
    return _cache["nc"]


def kernel(**inputs):
    import ml_dtypes
    bf16 = ml_dtypes.bfloat16
    f32 = np.float32

    x = np.asarray(inputs["x"], dtype=f32)
    y = np.asarray(inputs["y"], dtype=f32)
    Wq = np.asarray(inputs["Wq"], dtype=f32)
    Wkv = np.asarray(inputs["Wkv"], dtype=f32)
    Wproj = np.asarray(inputs["Wproj"], dtype=f32)
    bproj = np.asarray(inputs["bproj"], dtype=f32)
    bsr_np = np.asarray(inputs["bsr"], dtype=f32)
    Wsr = np.asarray(inputs["Wsr"], dtype=f32)
    gn = np.asarray(inputs["gn"], dtype=f32)
    bn = np.asarray(inputs["bn"], dtype=f32)

    w = {
        "WqT": np.ascontiguousarray(Wq.T).astype(bf16),
        "WsrT": np.ascontiguousarray(0.25 * Wsr.T).astype(bf16),
        "WkvT": np.ascontiguousarray(Wkv.T).astype(bf16),
        "WpT": np.ascontiguousarray(Wproj.T).astype(bf16),
        "bsr": bsr_np,
        "gnr": np.ascontiguousarray(gn.reshape(2, 128)).astype(f32),
        "bnc": bn,
        "bp": np.ascontiguousarray(bproj.reshape(1, C1)).astype(bf16),
    }

    # x: (B, 3136, 512) spatial-major -> (B, 512, 3136) window-major bf16,
    # one fused gather+cast. col n' = (wi*8+wj)*49 + i*7 + j.
    xw = x.reshape(B, 8, 7, 8, 7, C1).transpose(0, 5, 1, 3, 2, 4) \
          .reshape(B, C1, N1).astype(bf16)
    # y: (B, 12544, 256) -> 2x2 sum-pool -> (B, 256, 3136) window-major bf16
    s1 = y.reshape(B, H2, HP, 2, C2).sum(3)
    s2 = s1.reshape(B, HP, 2, HP, C2).sum(2)          # (B, 56, 56, 256)
    ypw = s2.reshape(B, 8, 7, 8, 7, C2).transpose(0, 5, 1, 3, 2, 4) \
            .reshape(B, C2, N1).astype(bf16)

    nc = _get_nc(w)
    in_maps = [{"xT": xw[b], "ypT": ypw[b]} for b in range(B)]
    from concourse.bass_utils import run_bass_kernel_spmd
    res = run_bass_kernel_spmd(nc, in_maps, core_ids=list(range(B)),
                               **_cache.get("run_opts", {}))
    _cache["last_res"] = res

    # dequantize: row n = nt*128 + p  ->  scale scl[p, nt] / 127
    i8 = np.stack([r["out"] for r in res.results], axis=0)      # (B, N1, C1) int8
    sc = np.stack([r["scl"] for r in res.results], axis=0)      # (B, 128, NT) f32
    srow = sc.transpose(0, 2, 1).reshape(B, NT * 128)[:, :N1] * (1.0 / 127.0)
    return i8 * srow[:, :, None]


# revision 10
# speedup vs baseline: 3.5995x; 1.3498x over previous
"""Trainium2 Bass kernel for windowed cross-attention (nn_CrossAttention_37056977830404).

Sharding: data-parallel over batch B=8 across the 8 NeuronCores (one batch
element per core). The call is transfer-bound over the axon tunnel
(~40 MB/s in, ~28 MB/s out), so the design minimizes bytes moved:
  - weights are baked into the NEFF as Const tensors (loaded once),
  - y is 2x2 sum-pooled on the host (4x reduction),
  - x and pooled-y are sent as int8 with per-row scales (2x vs bf16),
  - the output is int8 with per-row scales (4x smaller than f32, and the
    donated zero output buffers the PJRT path uploads shrink the same way).

Per-core pipeline (all shapes hardcoded):
  stage 0: xN [3136,512] i8, ypN [3136,256] i8 (natural layout) are
  dequantized to bf16 (per-row scales) and transposed on the tensor engine
  (identity matmuls, 56-row chunks) into window-major channel-major SBUF
  tiles: col n' = (wi*8+wj)*49 + i*7 + j.
  z = yp @ Wsr.T + bsr  (bf16 matmul, fp32 psum)     [sr conv; /4 in scales]
  LN over channels (cross-partition ones-matmul sums) + gelu -> y2T bf16
  kT = (y2 @ Wkv_k.T).T     [channel-major, bf16]
  v_w = y2 @ Wkv_v.T        [window-major via windowed stationary APs, bf16]
  qT = (x @ Wq.T).T         [channel-major, bf16]
  per (head, window-row): S^T = k_w^T q_w ; E = exp(S^T/8) ; sums via
  ones-matmul broadcast ; AV = v_w^T E ; attT = AV * recip(sum)  [bf16]
  out = attT.T @ Wproj.T + bproj  (bf16 matmuls), then per-row int8
  quantization: scl = absmax(row), out_i8 = round(out * 127/scl).
"""
import os
import sys

sys.path.insert(0, '/opt/trn_rl_repo')
os.environ.setdefault("JAX_COMPILATION_CACHE_DIR", "/tmp/jax_ccache")
os.environ.setdefault("JAX_PERSISTENT_CACHE_MIN_COMPILE_TIME_SECS", "0")
os.environ.setdefault("JAX_PERSISTENT_CACHE_MIN_ENTRY_SIZE_BYTES", "0")
import numpy as np

try:  # env vars above are too late if jax was imported first; force via config
    import jax as _jax
    _jax.config.update("jax_compilation_cache_dir", "/tmp/jax_ccache")
    _jax.config.update("jax_persistent_cache_min_compile_time_secs", 0)
    _jax.config.update("jax_persistent_cache_min_entry_size_bytes", 0)
except Exception:
    pass

B = 8
C1 = 512
N1 = 3136
NH = 8
HD = 64
WS = 7
C2 = 256
H2 = W2 = 112
HP = WP = 56
NCH = 392      # dense matmul n-chunk (free dim) = one window-row
NCHUNKS = 8    # 3136 / 392
NT = 25        # output row tiles (24x128 + 64)
EPS = 1e-5

_cache = {}


def _build_nc(w):
    import concourse.bacc as bacc
    import concourse.tile as tile
    from concourse import mybir

    F32 = mybir.dt.float32
    BF16 = mybir.dt.bfloat16
    I8 = mybir.dt.int8

    nc = bacc.Bacc()

    # ---------------- DRAM I/O ----------------
    xN = nc.dram_tensor("xN", [N1, C1], I8, kind="ExternalInput")
    xs = nc.dram_tensor("xs", [N1, 1], F32, kind="ExternalInput")
    ypN = nc.dram_tensor("ypN", [N1, C2], I8, kind="ExternalInput")
    yps = nc.dram_tensor("yps", [N1, 1], F32, kind="ExternalInput")
    out = nc.dram_tensor("out", [N1, C1], I8, kind="ExternalOutput")
    scl = nc.dram_tensor("scl", [128, NT], F32, kind="ExternalOutput")
    # weights baked into the NEFF (DMA'd to HBM once at model load)
    consts = {
        "WqT": nc.inline_tensor(w["WqT"], name="cWqT"),
        "WsrT": nc.inline_tensor(w["WsrT"], name="cWsrT"),
        "WkvT": nc.inline_tensor(w["WkvT"], name="cWkvT"),
        "WpT": nc.inline_tensor(w["WpT"], name="cWpT"),
        "bsr": nc.inline_tensor(w["bsr"], name="cbsr"),
        "gnr": nc.inline_tensor(w["gnr"], name="cgnr"),
        "bnc": nc.inline_tensor(w["bnc"], name="cbnc"),
        "bp": nc.inline_tensor(w["bp"], name="cbp"),
        "eye": nc.inline_tensor(w["eye"], name="ceye"),
    }

    with tile.TileContext(nc) as tc:
        _emit(nc, tc, mybir, F32, BF16, I8,
              xN, xs, ypN, yps, consts, out, scl)
    nc.finalize()
    return nc


def _emit(nc, tc, mybir, F32, BF16, I8, xN, xs, ypN, yps, consts, out, scl):
    from contextlib import ExitStack

    F32R = mybir.dt.float32r
    AF = mybir.ActivationFunctionType
    WqT, WsrT, WkvT, WpT = (consts["WqT"], consts["WsrT"], consts["WkvT"],
                            consts["WpT"])
    bsr, gnr, bnc, bp = consts["bsr"], consts["gnr"], consts["bnc"], consts["bp"]

    with ExitStack() as ctx:
        pool_w = ctx.enter_context(tc.tile_pool(name="pool_w", bufs=1))
        pool_big = ctx.enter_context(tc.tile_pool(name="pool_big", bufs=1))
        pool_vw = ctx.enter_context(tc.tile_pool(name="pool_vw", bufs=2))
        pool_tmp = ctx.enter_context(tc.tile_pool(name="pool_tmp", bufs=2))

        # ---------------- weights / constants to SBUF ----------------
        wq, wp, wsr, wkv = [], [], [], []
        for ct in range(4):
            wq_t = pool_w.tile([128, C1], BF16, name=f"wq{ct}", tag=f"wq{ct}")
            nc.sync.dma_start(out=wq_t, in_=WqT[ct * 128:(ct + 1) * 128, :])
            wq.append(wq_t)
            wp_t = pool_w.tile([128, C1], BF16, name=f"wp{ct}", tag=f"wp{ct}")
            nc.sync.dma_start(out=wp_t, in_=WpT[ct * 128:(ct + 1) * 128, :])
            wp.append(wp_t)
        for kt in range(2):
            wsr_t = pool_w.tile([128, C2], BF16, name=f"wsr{kt}", tag=f"wsr{kt}")
            nc.sync.dma_start(out=wsr_t, in_=WsrT[kt * 128:(kt + 1) * 128, :])
            wsr.append(wsr_t)
            wkv_t = pool_w.tile([128, 2 * C1], BF16, name=f"wkv{kt}", tag=f"wkv{kt}")
            nc.sync.dma_start(out=wkv_t, in_=WkvT[kt * 128:(kt + 1) * 128, :])
            wkv.append(wkv_t)
        bsr_c, bn_c, gn_r = [], [], []
        for ot in range(2):
            b1 = pool_w.tile([128, 1], F32, name=f"bsr{ot}", tag=f"bsr{ot}")
            nc.sync.dma_start(out=b1, in_=bsr[ot * 128:(ot + 1) * 128].unsqueeze(1))
            bsr_c.append(b1)
            b2 = pool_w.tile([128, 1], F32, name=f"bn{ot}", tag=f"bn{ot}")
            nc.sync.dma_start(out=b2, in_=bnc[ot * 128:(ot + 1) * 128].unsqueeze(1))
            bn_c.append(b2)
            g0 = pool_w.tile([1, 128], F32, name=f"gnrf{ot}", tag=f"gnrf{ot}")
            nc.sync.dma_start(out=g0, in_=gnr[ot:ot + 1, :])
            g1 = pool_w.tile([1, 128], F32R, name=f"gnr{ot}", tag=f"gnr{ot}")
            nc.vector.tensor_copy(g1[:], g0[:])
            gn_r.append(g1)
        bp_sb = pool_w.tile([1, C1], BF16, name="bp_sb", tag="bp_sb")
        nc.sync.dma_start(out=bp_sb, in_=bp.ap())
        eye_sb = pool_w.tile([HP, HP], BF16, name="eye_sb", tag="eye_sb")
        nc.sync.dma_start(out=eye_sb, in_=consts["eye"].ap())
        # per-row dequant scales, transposed load: xs_all[p, r] = xs[r*56+p]
        xs_all = pool_w.tile([HP, HP], F32, name="xs_all", tag="xs_all")
        nc.sync.dma_start(out=xs_all,
                          in_=xs.ap().rearrange("(r p) one -> p (r one)",
                                                r=HP, p=HP))
        ys_all = pool_w.tile([HP, HP], F32, name="ys_all", tag="ys_all")
        nc.sync.dma_start(out=ys_all,
                          in_=yps.ap().rearrange("(r p) one -> p (r one)",
                                                 r=HP, p=HP))

        ones_f = pool_w.tile([128, 1], F32, name="ones_f", tag="ones_f")
        nc.vector.memset(ones_f, 1.0)
        ones_c = pool_w.tile([128, 1], F32R, name="ones_c", tag="ones_c")
        nc.vector.tensor_copy(ones_c[:], ones_f[:])
        ones_rf = pool_w.tile([1, 128], F32, name="ones_rf", tag="ones_rf")
        nc.vector.memset(ones_rf, 1.0)
        ones_r = pool_w.tile([1, 128], BF16, name="ones_r", tag="ones_r")
        nc.vector.tensor_copy(ones_r[:], ones_rf[:])
        ones_s = pool_w.tile([49, 64], BF16, name="ones_s", tag="ones_s")
        nc.vector.memset(ones_s, 1.0)
        eps_sb = pool_w.tile([1, 1], F32, name="eps_sb", tag="eps_sb")
        nc.vector.memset(eps_sb, EPS)

        # ---------------- persistent activations ----------------
        xT = [pool_big.tile([128, N1], BF16, name=f"xT{t}", tag=f"xT{t}")
              for t in range(4)]
        y2T = [pool_big.tile([128, N1], BF16, name=f"y2T{k}", tag=f"y2T{k}")
               for k in range(2)]
        kT = [pool_big.tile([128, N1], BF16, name=f"kT{t}", tag=f"kT{t}")
              for t in range(4)]
        qT = [pool_big.tile([128, N1], BF16, name=f"qT{t}", tag=f"qT{t}")
              for t in range(4)]

        def wdest(tile_, r):
            # window-major scatter view for spatial row r: [p, wj 8, j 7]
            # target col = (wi*8+wj)*49 + i*7 + j
            wi, i = r // WS, r % WS
            v = tile_.rearrange("p (a b i j) -> p a b i j", a=8, b=8, i=7, j=7)
            return v[:, wi, :, i]

        with tc.tile_pool(name="pool_yp", bufs=1) as pool_yp:
            ypT = [pool_yp.tile([128, N1], BF16, name=f"ypT{k}", tag=f"ypT{k}")
                   for k in range(2)]

            # ------------ stage 0: dequant + transpose to window-major ------------
            with tc.tile_pool(name="ps_t", bufs=1, space="PSUM") as ps_t:
                for r in range(HP):
                    rs_ = slice(r * HP, (r + 1) * HP)
                    xi = pool_tmp.tile([HP, C1], I8, name="xi", tag="xi", bufs=3)
                    nc.sync.dma_start(out=xi, in_=xN[rs_, :])
                    xb = pool_tmp.tile([HP, C1], BF16, name="xb", tag="xb", bufs=3)
                    nc.scalar.activation(out=xb[:], in_=xi[:], func=AF.Identity,
                                         scale=xs_all[:, r:r + 1])
                    yi = pool_tmp.tile([HP, C2], I8, name="yi", tag="yi", bufs=3)
                    nc.sync.dma_start(out=yi, in_=ypN[rs_, :])
                    yb = pool_tmp.tile([HP, C2], BF16, name="yb", tag="yb", bufs=3)
                    nc.scalar.activation(out=yb[:], in_=yi[:], func=AF.Identity,
                                         scale=ys_all[:, r:r + 1])
                    for cb in range(4):
                        pt = ps_t.tile([128, HP], F32, name="pt", tag="pt", bufs=6)
                        nc.tensor.matmul(pt[:], xb[:, cb * 128:(cb + 1) * 128],
                                         eye_sb[:], start=True, stop=True)
                        nc.vector.tensor_copy(
                            wdest(xT[cb], r),
                            pt.rearrange("p (b j) -> p b j", b=8, j=7))
                    for cb in range(2):
                        pt = ps_t.tile([128, HP], F32, name="pt2", tag="pt", bufs=6)
                        nc.tensor.matmul(pt[:], yb[:, cb * 128:(cb + 1) * 128],
                                         eye_sb[:], start=True, stop=True)
                        nc.vector.tensor_copy(
                            wdest(ypT[cb], r),
                            pt.rearrange("p (b j) -> p b j", b=8, j=7))

            ps_d_cm = tc.tile_pool(name="ps_d", bufs=2, space="PSUM")
            ps_d = ps_d_cm.__enter__()
            # ------------ stage 2: sr conv + LN + gelu ------------
            for ch in range(NCHUNKS):
                cs = slice(ch * NCH, (ch + 1) * NCH)
                zsb = []
                for ot in range(2):
                    pz = ps_d.tile([128, NCH], F32, name="pz", tag="pz")
                    for kt in range(2):
                        nc.tensor.matmul(pz[:], wsr[kt][:, ot * 128:(ot + 1) * 128],
                                         ypT[kt][:, cs],
                                         start=(kt == 0), stop=(kt == 1))
                    z_t = pool_tmp.tile([128, NCH], F32R, name="z_t",
                                        tag="zsb", bufs=4)
                    nc.scalar.activation(out=z_t[:], in_=pz[:], func=AF.Identity,
                                         bias=bsr_c[ot])
                    zsb.append(z_t)
                pst_s = ps_d.tile([1, NCH], F32, name="pst_s", tag="pst_s", bufs=1)
                pst_q = ps_d.tile([1, NCH], F32, name="pst_q", tag="pst_q", bufs=1)
                for ot in range(2):
                    nc.tensor.matmul(pst_s[:], ones_c[:], zsb[ot][:],
                                     start=(ot == 0), stop=(ot == 1))
                for ot in range(2):
                    zq = pool_tmp.tile([128, NCH], F32R, name="zq", tag="zq", bufs=2)
                    nc.scalar.activation(out=zq[:], in_=zsb[ot][:], func=AF.Square)
                    nc.tensor.matmul(pst_q[:], ones_c[:], zq[:],
                                     start=(ot == 0), stop=(ot == 1))
                m_sb = pool_tmp.tile([1, NCH], F32, name="m_sb", tag="m_sb", bufs=1)
                nc.vector.tensor_scalar_mul(m_sb[:], pst_s[:], 1.0 / C2)
                q_sb = pool_tmp.tile([1, NCH], F32, name="q_sb", tag="q_sb", bufs=1)
                nc.vector.tensor_scalar_mul(q_sb[:], pst_q[:], 1.0 / C2)
                var_sb = pool_tmp.tile([1, NCH], F32, name="var_sb",
                                       tag="var_sb", bufs=1)
                nc.gpsimd.tensor_tensor(var_sb[:], m_sb[:], m_sb[:],
                                        op=mybir.AluOpType.mult)
                nc.gpsimd.tensor_tensor(var_sb[:], q_sb[:], var_sb[:],
                                        op=mybir.AluOpType.subtract)
                sd_sb = pool_tmp.tile([1, NCH], F32, name="sd_sb",
                                      tag="sd_sb", bufs=1)
                nc.scalar.activation(out=sd_sb[:], in_=var_sb[:], func=AF.Sqrt,
                                     bias=eps_sb[:])
                r_sb = pool_tmp.tile([1, NCH], F32R, name="r_sb", tag="r_sb", bufs=1)
                with nc.allow_low_precision(reason="f32r rstd feeds f32r matmul"):
                    nc.vector.reciprocal(out=r_sb[:], in_=sd_sb[:])
                nb_sb = pool_tmp.tile([1, NCH], F32R, name="nb_sb",
                                      tag="nb_sb", bufs=1)
                nc.gpsimd.tensor_tensor(nb_sb[:], m_sb[:], r_sb[:],
                                        op=mybir.AluOpType.mult)
                nc.gpsimd.tensor_scalar_mul(nb_sb[:], nb_sb[:], -1.0)
                for ot in range(2):
                    pa = ps_d.tile([128, NCH], F32, name="pa", tag="pa")
                    nc.tensor.matmul(pa[:], gn_r[ot][:], r_sb[:],
                                     start=True, stop=True)
                    pb = ps_d.tile([128, NCH], F32, name="pb", tag="pb")
                    nc.tensor.matmul(pb[:], gn_r[ot][:], nb_sb[:],
                                     start=True, stop=True)
                    t1 = pool_tmp.tile([128, NCH], F32, name="t1", tag="t1", bufs=2)
                    nc.vector.tensor_mul(t1[:], zsb[ot][:], pa[:])
                    nc.vector.tensor_add(t1[:], t1[:], pb[:])
                    nc.scalar.activation(out=y2T[ot][:, cs], in_=t1[:],
                                         func=AF.Gelu, bias=bn_c[ot])

            # ------------ stage 3: k projection (channel-major) ------------
            for ch in range(NCHUNKS):
                cs = slice(ch * NCH, (ch + 1) * NCH)
                for ot in range(4):
                    pk = ps_d.tile([128, NCH], F32, name="pk", tag="pz")
                    for kt in range(2):
                        nc.tensor.matmul(pk[:],
                                         wkv[kt][:, ot * 128:(ot + 1) * 128],
                                         y2T[kt][:, cs],
                                         start=(kt == 0), stop=(kt == 1))
                    nc.any.tensor_copy(kT[ot][:, cs], pk[:])

            # ------------ stage 4: q projection (channel-major) ------------
            for ch in range(NCHUNKS):
                cs = slice(ch * NCH, (ch + 1) * NCH)
                for ot in range(4):
                    pq = ps_d.tile([128, NCH], F32, name="pq", tag="pz")
                    for ct in range(4):
                        nc.tensor.matmul(pq[:],
                                         wq[ct][:, ot * 128:(ot + 1) * 128],
                                         xT[ct][:, cs],
                                         start=(ct == 0), stop=(ct == 3))
                    nc.any.tensor_copy(qT[ot][:, cs], pq[:])
            ps_d_cm.__exit__(None, None, None)

        # ------------ stage 5-7: v (window-major), attention, proj ------------
        # qT/kT/y2T columns are window-major: window w = wi*8+wj occupies
        # cols w*49:(w+1)*49. attT stays spatial-major (scatter on write).

        def win_view(t):
            return t.rearrange("p (a i b j) -> p a b i j", a=8, i=7, b=8, j=7)

        with tc.tile_pool(name="pool_att", bufs=1) as pool_att, \
             tc.tile_pool(name="ps_a", bufs=2, space="PSUM") as ps_a:
            attT = [pool_att.tile([128, N1], BF16, name=f"attT{t}", tag=f"attT{t}")
                    for t in range(4)]
            for wi in range(8):
                vw = pool_vw.tile([49, 8 * C1], BF16, name="vw", tag="vw")
                for wj in range(8):
                    wsl = slice((wi * 8 + wj) * 49, (wi * 8 + wj + 1) * 49)
                    pv = ps_a.tile([49, C1], F32, name="pv", tag="pv")
                    for kt in range(2):
                        nc.tensor.matmul(pv[:], y2T[kt][:, wsl],
                                         wkv[kt][:, C1:2 * C1],
                                         start=(kt == 0), stop=(kt == 1))
                    nc.scalar.copy(out=vw[:, wj * C1:(wj + 1) * C1], in_=pv[:])
                for h in range(8):
                    t, pb_ = h // 2, (h % 2) * 64
                    psl = slice(pb_, pb_ + 64)
                    S = ps_a.tile([49, 392], F32, name="S", tag="S")
                    for wj in range(8):
                        wsl = slice((wi * 8 + wj) * 49, (wi * 8 + wj + 1) * 49)
                        nc.tensor.matmul(S[:, wj * 49:(wj + 1) * 49],
                                         kT[t][psl, wsl],
                                         qT[t][psl, wsl],
                                         start=True, stop=True)
                    E = pool_tmp.tile([49, 392], BF16, name="E", tag="E", bufs=3)
                    nc.scalar.activation(out=E[:], in_=S[:], func=AF.Exp,
                                         scale=0.125)
                    SUMB = ps_a.tile([64, 392], F32, name="SUMB",
                                     tag="SUMB", bufs=1)
                    nc.tensor.matmul(SUMB[:], ones_s[:], E[:],
                                     start=True, stop=True)
                    RB = pool_tmp.tile([64, 392], F32, name="RB", tag="RB", bufs=3)
                    nc.vector.reciprocal(out=RB[:], in_=SUMB[:])
                    AV = ps_a.tile([64, 392], F32, name="AV", tag="AV")
                    for wj in range(8):
                        nc.tensor.matmul(
                            AV[:, wj * 49:(wj + 1) * 49],
                            vw[:, wj * C1 + h * 64:wj * C1 + (h + 1) * 64],
                            E[:, wj * 49:(wj + 1) * 49],
                            start=True, stop=True)
                    avv = AV.rearrange("p (b i j) -> p b i j", b=8, i=7, j=7)
                    rbv = RB.rearrange("p (b i j) -> p b i j", b=8, i=7, j=7)
                    nc.vector.tensor_mul(win_view(attT[t])[psl, wi],
                                         avv[:], rbv[:])

            # ------------ stage 7: output projection + int8 quantization ------------
            for nt in range(NT):
                nsz = min(128, N1 - nt * 128)
                ns = slice(nt * 128, nt * 128 + nsz)
                po = ps_a.tile([128, C1], F32, name="po", tag="pv")
                for ct in range(4):
                    nc.tensor.matmul(po[:nsz, :], attT[ct][:, ns], wp[ct][:],
                                     start=(ct == 0), stop=False)
                nc.tensor.matmul(po[:nsz, :], ones_r[:, :nsz], bp_sb[:],
                                 start=False, stop=True)
                # per-row absmax -> int8 scale; conversion rounds-to-nearest
                mx = pool_tmp.tile([128, 1], F32, name="mx", tag="mx", bufs=2)
                nc.vector.tensor_reduce(mx[:nsz, :], po[:nsz, :],
                                        axis=mybir.AxisListType.X,
                                        op=mybir.AluOpType.max,
                                        apply_absolute_value=True)
                nc.vector.tensor_scalar_max(mx[:nsz, :], mx[:nsz, :], 1e-30)
                rs = pool_tmp.tile([128, 1], F32, name="rs", tag="rs", bufs=2)
                nc.vector.reciprocal(out=rs[:nsz, :], in_=mx[:nsz, :])
                nc.vector.tensor_scalar_mul(rs[:nsz, :], rs[:nsz, :], 127.0)
                o_i8 = pool_tmp.tile([128, C1], I8, name="o_i8",
                                     tag="o_i8", bufs=2)
                nc.scalar.activation(out=o_i8[:nsz, :], in_=po[:nsz, :],
                                     func=AF.Identity, scale=rs[:nsz, :])
                nc.sync.dma_start(out=out[ns, :], in_=o_i8[:nsz, :])
                nc.sync.dma_start(out=scl[:nsz, nt:nt + 1], in_=mx[:nsz, :])


def _get_nc(w):
    rebuild = True
    if "nc" in _cache:
        old = _cache["w"]
        rebuild = not all(np.array_equal(old[k], w[k]) for k in old)
    if rebuild:
        _cache["nc"] = _build_nc(w)
        _cache["w"] = w
    return _cache["nc"]


def _quant_rows(a, pow2=1.0):
    """Per-row symmetric int8: returns (int8 array, dequant scale per row)."""
    s = np.abs(a).max(-1, keepdims=True)
    np.maximum(s, 1e-30, out=s)
    t = a * (127.0 / s)
    t += 128.5
    u = t.astype(np.uint8)
    np.bitwise_xor(u, 128, out=u)
    return u.view(np.int8), (s * (1.0 / (127.0 * pow2))).astype(np.float32)


def kernel(**inputs):
    import ml_dtypes
    bf16 = ml_dtypes.bfloat16
    f32 = np.float32

    x = np.asarray(inputs["x"], dtype=f32)
    y = np.asarray(inputs["y"], dtype=f32)
    Wq = np.asarray(inputs["Wq"], dtype=f32)
    Wkv = np.asarray(inputs["Wkv"], dtype=f32)
    Wproj = np.asarray(inputs["Wproj"], dtype=f32)
    bproj = np.asarray(inputs["bproj"], dtype=f32)
    bsr_np = np.asarray(inputs["bsr"], dtype=f32)
    Wsr = np.asarray(inputs["Wsr"], dtype=f32)
    gn = np.asarray(inputs["gn"], dtype=f32)
    bn = np.asarray(inputs["bn"], dtype=f32)

    w = {
        "WqT": np.ascontiguousarray(Wq.T).astype(bf16),
        "WsrT": np.ascontiguousarray(Wsr.T).astype(bf16),
        "WkvT": np.ascontiguousarray(Wkv.T).astype(bf16),
        "WpT": np.ascontiguousarray(Wproj.T).astype(bf16),
        "bsr": bsr_np,
        "gnr": np.ascontiguousarray(gn.reshape(2, 128)).astype(f32),
        "bnc": bn,
        "bp": np.ascontiguousarray(bproj.reshape(1, C1)).astype(bf16),
        "eye": np.eye(HP, dtype=bf16),
    }

    # x: per-row int8 (natural layout; device transposes + window-majors)
    xi8, xsc = _quant_rows(x.reshape(B, N1, C1))
    # y: 2x2 sum-pool on host, per-row int8; /4 (pool mean) folded into scales
    s1 = y.reshape(B, H2, HP, 2, C2).sum(3)
    ysum = s1.reshape(B, HP, 2, HP, C2).sum(2).reshape(B, N1, C2)
    yi8, ysc = _quant_rows(ysum, pow2=4.0)

    nc = _get_nc(w)
    in_maps = [{"xN": xi8[b], "xs": xsc[b], "ypN": yi8[b], "yps": ysc[b]}
               for b in range(B)]
    from concourse.bass_utils import run_bass_kernel_spmd
    res = run_bass_kernel_spmd(nc, in_maps, core_ids=list(range(B)),
                               **_cache.get("run_opts", {}))
    _cache["last_res"] = res

    # dequantize: row n = nt*128 + p  ->  scale scl[p, nt] / 127
    i8 = np.stack([r["out"] for r in res.results], axis=0)      # (B, N1, C1) int8
    sc = np.stack([r["scl"] for r in res.results], axis=0)      # (B, 128, NT) f32
    srow = sc.transpose(0, 2, 1).reshape(B, NT * 128)[:, :N1] * (1.0 / 127.0)
    return i8 * srow[:, :, None]


# revision 13
# speedup vs baseline: 3.6341x; 1.0096x over previous
"""Trainium2 Bass kernel for windowed cross-attention (nn_CrossAttention_37056977830404).

Sharding: data-parallel over batch B=8 across the 8 NeuronCores (one batch
element per core). The call is transfer-bound over the axon tunnel
(~40 MB/s in, ~28 MB/s out), so the design minimizes bytes moved:
  - weights are baked into the NEFF as Const tensors (loaded once),
  - y is 2x2 sum-pooled on the host (4x reduction),
  - x and pooled-y are sent as int8 with per-row scales (2x vs bf16),
  - the output is int8 with per-row scales (4x smaller than f32, and the
    donated zero output buffers the PJRT path uploads shrink the same way).

Per-core pipeline (all shapes hardcoded):
  stage 0: xN [3136,512] i8, ypN [3136,256] i8 (natural layout) are
  dequantized to bf16 (per-row scales) and transposed on the tensor engine
  (identity matmuls, 56-row chunks) into window-major channel-major SBUF
  tiles: col n' = (wi*8+wj)*49 + i*7 + j.
  z = yp @ Wsr.T + bsr  (bf16 matmul, fp32 psum)     [sr conv; /4 in scales]
  LN over channels (cross-partition ones-matmul sums) + gelu -> y2T bf16
  kT = (y2 @ Wkv_k.T).T     [channel-major, bf16]
  v_w = y2 @ Wkv_v.T        [window-major via windowed stationary APs, bf16]
  qT = (x @ Wq.T).T         [channel-major, bf16]
  per (head, window-row): S^T = k_w^T q_w ; E = exp(S^T/8) ; sums via
  ones-matmul broadcast ; AV = v_w^T E ; attT = AV * recip(sum)  [bf16]
  out = attT.T @ Wproj.T + bproj  (bf16 matmuls), then per-row int8
  quantization: scl = absmax(row), out_i8 = round(out * 127/scl).
"""
import os
import sys

sys.path.insert(0, '/opt/trn_rl_repo')
os.environ.setdefault("JAX_COMPILATION_CACHE_DIR", "/tmp/jax_ccache")
os.environ.setdefault("JAX_PERSISTENT_CACHE_MIN_COMPILE_TIME_SECS", "0")
os.environ.setdefault("JAX_PERSISTENT_CACHE_MIN_ENTRY_SIZE_BYTES", "0")
import numpy as np

try:  # env vars above are too late if jax was imported first; force via config
    import jax as _jax
    _jax.config.update("jax_compilation_cache_dir", "/tmp/jax_ccache")
    _jax.config.update("jax_persistent_cache_min_compile_time_secs", 0)
    _jax.config.update("jax_persistent_cache_min_entry_size_bytes", 0)
except Exception:
    pass

B = 8
C1 = 512
N1 = 3136
NH = 8
HD = 64
WS = 7
C2 = 256
H2 = W2 = 112
HP = WP = 56
NCH = 392      # dense matmul n-chunk (free dim) = one window-row
NCHUNKS = 8    # 3136 / 392
NT = 25        # output row tiles (24x128 + 64)
EPS = 1e-5

_cache = {}


def _build_nc(w):
    import concourse.bacc as bacc
    import concourse.tile as tile
    from concourse import mybir

    F32 = mybir.dt.float32
    BF16 = mybir.dt.bfloat16
    I8 = mybir.dt.int8

    nc = bacc.Bacc()

    # ---------------- DRAM I/O ----------------
    xN = nc.dram_tensor("xN", [N1, C1], I8, kind="ExternalInput")
    xs = nc.dram_tensor("xs", [N1, 1], F32, kind="ExternalInput")
    ypN = nc.dram_tensor("ypN", [N1, C2], I8, kind="ExternalInput")
    yps = nc.dram_tensor("yps", [N1, 1], F32, kind="ExternalInput")
    out = nc.dram_tensor("out", [N1, C1], I8, kind="ExternalOutput")
    scl = nc.dram_tensor("scl", [128, NT], F32, kind="ExternalOutput")
    # weights baked into the NEFF (DMA'd to HBM once at model load)
    consts = {
        "WqT": nc.inline_tensor(w["WqT"], name="cWqT"),
        "WsrT": nc.inline_tensor(w["WsrT"], name="cWsrT"),
        "WkvT": nc.inline_tensor(w["WkvT"], name="cWkvT"),
        "WpT": nc.inline_tensor(w["WpT"], name="cWpT"),
        "bsr": nc.inline_tensor(w["bsr"], name="cbsr"),
        "gnr": nc.inline_tensor(w["gnr"], name="cgnr"),
        "bnc": nc.inline_tensor(w["bnc"], name="cbnc"),
        "bp": nc.inline_tensor(w["bp"], name="cbp"),
        "eye": nc.inline_tensor(w["eye"], name="ceye"),
    }

    with tile.TileContext(nc) as tc:
        _emit(nc, tc, mybir, F32, BF16, I8,
              xN, xs, ypN, yps, consts, out, scl)
    nc.finalize()
    return nc


def _emit(nc, tc, mybir, F32, BF16, I8, xN, xs, ypN, yps, consts, out, scl):
    from contextlib import ExitStack

    F32R = mybir.dt.float32r
    AF = mybir.ActivationFunctionType
    WqT, WsrT, WkvT, WpT = (consts["WqT"], consts["WsrT"], consts["WkvT"],
                            consts["WpT"])
    bsr, gnr, bnc, bp = consts["bsr"], consts["gnr"], consts["bnc"], consts["bp"]

    with ExitStack() as ctx:
        pool_w = ctx.enter_context(tc.tile_pool(name="pool_w", bufs=1))
        pool_big = ctx.enter_context(tc.tile_pool(name="pool_big", bufs=1))
        pool_vw = ctx.enter_context(tc.tile_pool(name="pool_vw", bufs=2))
        pool_tmp = ctx.enter_context(tc.tile_pool(name="pool_tmp", bufs=2))

        # ---------------- weights / constants to SBUF ----------------
        wq, wp, wsr, wkv = [], [], [], []
        for ct in range(4):
            wq_t = pool_w.tile([128, C1], BF16, name=f"wq{ct}", tag=f"wq{ct}")
            nc.sync.dma_start(out=wq_t, in_=WqT[ct * 128:(ct + 1) * 128, :])
            wq.append(wq_t)
            wp_t = pool_w.tile([128, C1], BF16, name=f"wp{ct}", tag=f"wp{ct}")
            nc.sync.dma_start(out=wp_t, in_=WpT[ct * 128:(ct + 1) * 128, :])
            wp.append(wp_t)
        for kt in range(2):
            wsr_t = pool_w.tile([128, C2], BF16, name=f"wsr{kt}", tag=f"wsr{kt}")
            nc.sync.dma_start(out=wsr_t, in_=WsrT[kt * 128:(kt + 1) * 128, :])
            wsr.append(wsr_t)
            wkv_t = pool_w.tile([128, 2 * C1], BF16, name=f"wkv{kt}", tag=f"wkv{kt}")
            nc.sync.dma_start(out=wkv_t, in_=WkvT[kt * 128:(kt + 1) * 128, :])
            wkv.append(wkv_t)
        bsr_c, bn_c, gn_r = [], [], []
        for ot in range(2):
            b1 = pool_w.tile([128, 1], F32, name=f"bsr{ot}", tag=f"bsr{ot}")
            nc.sync.dma_start(out=b1, in_=bsr[ot * 128:(ot + 1) * 128].unsqueeze(1))
            bsr_c.append(b1)
            b2 = pool_w.tile([128, 1], F32, name=f"bn{ot}", tag=f"bn{ot}")
            nc.sync.dma_start(out=b2, in_=bnc[ot * 128:(ot + 1) * 128].unsqueeze(1))
            bn_c.append(b2)
            g0 = pool_w.tile([1, 128], F32, name=f"gnrf{ot}", tag=f"gnrf{ot}")
            nc.sync.dma_start(out=g0, in_=gnr[ot:ot + 1, :])
            g1 = pool_w.tile([1, 128], F32R, name=f"gnr{ot}", tag=f"gnr{ot}")
            nc.vector.tensor_copy(g1[:], g0[:])
            gn_r.append(g1)
        bp_sb = pool_w.tile([1, C1], BF16, name="bp_sb", tag="bp_sb")
        nc.sync.dma_start(out=bp_sb, in_=bp.ap())
        eye_sb = pool_w.tile([HP, HP], BF16, name="eye_sb", tag="eye_sb")
        nc.sync.dma_start(out=eye_sb, in_=consts["eye"].ap())
        # per-row dequant scales, transposed load: xs_all[p, r] = xs[r*56+p]
        xs_all = pool_w.tile([HP, HP], F32, name="xs_all", tag="xs_all")
        nc.sync.dma_start(out=xs_all,
                          in_=xs.ap().rearrange("(r p) one -> p (r one)",
                                                r=HP, p=HP))
        ys_all = pool_w.tile([HP, HP], F32, name="ys_all", tag="ys_all")
        nc.sync.dma_start(out=ys_all,
                          in_=yps.ap().rearrange("(r p) one -> p (r one)",
                                                 r=HP, p=HP))

        ones_f = pool_w.tile([128, 1], F32, name="ones_f", tag="ones_f")
        nc.vector.memset(ones_f, 1.0)
        ones_c = pool_w.tile([128, 1], F32R, name="ones_c", tag="ones_c")
        nc.vector.tensor_copy(ones_c[:], ones_f[:])
        ones_rf = pool_w.tile([1, 128], F32, name="ones_rf", tag="ones_rf")
        nc.vector.memset(ones_rf, 1.0)
        ones_r = pool_w.tile([1, 128], BF16, name="ones_r", tag="ones_r")
        nc.vector.tensor_copy(ones_r[:], ones_rf[:])
        ones_s = pool_w.tile([49, 64], BF16, name="ones_s", tag="ones_s")
        nc.vector.memset(ones_s, 1.0)
        eps_sb = pool_w.tile([1, 1], F32, name="eps_sb", tag="eps_sb")
        nc.vector.memset(eps_sb, EPS)

        # ---------------- persistent activations ----------------
        xT = [pool_big.tile([128, N1], BF16, name=f"xT{t}", tag=f"xT{t}")
              for t in range(4)]
        y2T = [pool_big.tile([128, N1], BF16, name=f"y2T{k}", tag=f"y2T{k}")
               for k in range(2)]
        kT = [pool_big.tile([128, N1], BF16, name=f"kT{t}", tag=f"kT{t}")
              for t in range(4)]
        qT = [pool_big.tile([128, N1], BF16, name=f"qT{t}", tag=f"qT{t}")
              for t in range(4)]

        def wdest(tile_, r):
            # window-major scatter view for spatial row r: [p, wj 8, j 7]
            # target col = (wi*8+wj)*49 + i*7 + j
            wi, i = r // WS, r % WS
            v = tile_.rearrange("p (a b i j) -> p a b i j", a=8, b=8, i=7, j=7)
            return v[:, wi, :, i]

        with tc.tile_pool(name="pool_yp", bufs=1) as pool_yp:
            ypT = [pool_yp.tile([128, N1], BF16, name=f"ypT{k}", tag=f"ypT{k}")
                   for k in range(2)]

            # ------------ stage 0: dequant + transpose to window-major ------------
            with tc.tile_pool(name="ps_t", bufs=1, space="PSUM") as ps_t:
                for r in range(HP):
                    rs_ = slice(r * HP, (r + 1) * HP)
                    xi = pool_tmp.tile([HP, C1], I8, name="xi", tag="xi", bufs=3)
                    nc.sync.dma_start(out=xi, in_=xN[rs_, :])
                    xb = pool_tmp.tile([HP, C1], BF16, name="xb", tag="xb", bufs=3)
                    nc.scalar.activation(out=xb[:], in_=xi[:], func=AF.Identity,
                                         scale=xs_all[:, r:r + 1])
                    yi = pool_tmp.tile([HP, C2], I8, name="yi", tag="yi", bufs=3)
                    nc.sync.dma_start(out=yi, in_=ypN[rs_, :])
                    yb = pool_tmp.tile([HP, C2], BF16, name="yb", tag="yb", bufs=3)
                    nc.scalar.activation(out=yb[:], in_=yi[:], func=AF.Identity,
                                         scale=ys_all[:, r:r + 1])
                    for cb in range(4):
                        pt = ps_t.tile([128, HP], F32, name="pt", tag="pt", bufs=6)
                        nc.tensor.matmul(pt[:], xb[:, cb * 128:(cb + 1) * 128],
                                         eye_sb[:], start=True, stop=True)
                        nc.vector.tensor_copy(
                            wdest(xT[cb], r),
                            pt.rearrange("p (b j) -> p b j", b=8, j=7))
                    for cb in range(2):
                        pt = ps_t.tile([128, HP], F32, name="pt2", tag="pt", bufs=6)
                        nc.tensor.matmul(pt[:], yb[:, cb * 128:(cb + 1) * 128],
                                         eye_sb[:], start=True, stop=True)
                        nc.vector.tensor_copy(
                            wdest(ypT[cb], r),
                            pt.rearrange("p (b j) -> p b j", b=8, j=7))

            ps_d_cm = tc.tile_pool(name="ps_d", bufs=2, space="PSUM")
            ps_d = ps_d_cm.__enter__()
            # ------------ stage 2: sr conv + LN + gelu ------------
            for ch in range(NCHUNKS):
                cs = slice(ch * NCH, (ch + 1) * NCH)
                zsb = []
                for ot in range(2):
                    pz = ps_d.tile([128, NCH], F32, name="pz", tag="pz")
                    for kt in range(2):
                        nc.tensor.matmul(pz[:], wsr[kt][:, ot * 128:(ot + 1) * 128],
                                         ypT[kt][:, cs],
                                         start=(kt == 0), stop=(kt == 1))
                    z_t = pool_tmp.tile([128, NCH], F32R, name="z_t",
                                        tag="zsb", bufs=4)
                    nc.scalar.activation(out=z_t[:], in_=pz[:], func=AF.Identity,
                                         bias=bsr_c[ot])
                    zsb.append(z_t)
                pst_s = ps_d.tile([1, NCH], F32, name="pst_s", tag="pst_s", bufs=1)
                pst_q = ps_d.tile([1, NCH], F32, name="pst_q", tag="pst_q", bufs=1)
                for ot in range(2):
                    nc.tensor.matmul(pst_s[:], ones_c[:], zsb[ot][:],
                                     start=(ot == 0), stop=(ot == 1))
                for ot in range(2):
                    zq = pool_tmp.tile([128, NCH], F32R, name="zq", tag="zq", bufs=2)
                    nc.scalar.activation(out=zq[:], in_=zsb[ot][:], func=AF.Square)
                    nc.tensor.matmul(pst_q[:], ones_c[:], zq[:],
                                     start=(ot == 0), stop=(ot == 1))
                m_sb = pool_tmp.tile([1, NCH], F32, name="m_sb", tag="m_sb", bufs=1)
                nc.vector.tensor_scalar_mul(m_sb[:], pst_s[:], 1.0 / C2)
                q_sb = pool_tmp.tile([1, NCH], F32, name="q_sb", tag="q_sb", bufs=1)
                nc.vector.tensor_scalar_mul(q_sb[:], pst_q[:], 1.0 / C2)
                var_sb = pool_tmp.tile([1, NCH], F32, name="var_sb",
                                       tag="var_sb", bufs=1)
                nc.gpsimd.tensor_tensor(var_sb[:], m_sb[:], m_sb[:],
                                        op=mybir.AluOpType.mult)
                nc.gpsimd.tensor_tensor(var_sb[:], q_sb[:], var_sb[:],
                                        op=mybir.AluOpType.subtract)
                sd_sb = pool_tmp.tile([1, NCH], F32, name="sd_sb",
                                      tag="sd_sb", bufs=1)
                nc.scalar.activation(out=sd_sb[:], in_=var_sb[:], func=AF.Sqrt,
                                     bias=eps_sb[:])
                r_sb = pool_tmp.tile([1, NCH], F32R, name="r_sb", tag="r_sb", bufs=1)
                with nc.allow_low_precision(reason="f32r rstd feeds f32r matmul"):
                    nc.vector.reciprocal(out=r_sb[:], in_=sd_sb[:])
                nb_sb = pool_tmp.tile([1, NCH], F32R, name="nb_sb",
                                      tag="nb_sb", bufs=1)
                nc.gpsimd.tensor_tensor(nb_sb[:], m_sb[:], r_sb[:],
                                        op=mybir.AluOpType.mult)
                nc.gpsimd.tensor_scalar_mul(nb_sb[:], nb_sb[:], -1.0)
                for ot in range(2):
                    pa = ps_d.tile([128, NCH], F32, name="pa", tag="pa")
                    nc.tensor.matmul(pa[:], gn_r[ot][:], r_sb[:],
                                     start=True, stop=True)
                    pb = ps_d.tile([128, NCH], F32, name="pb", tag="pb")
                    nc.tensor.matmul(pb[:], gn_r[ot][:], nb_sb[:],
                                     start=True, stop=True)
                    t1 = pool_tmp.tile([128, NCH], F32, name="t1", tag="t1", bufs=2)
                    nc.vector.tensor_mul(t1[:], zsb[ot][:], pa[:])
                    nc.vector.tensor_add(t1[:], t1[:], pb[:])
                    nc.scalar.activation(out=y2T[ot][:, cs], in_=t1[:],
                                         func=AF.Gelu, bias=bn_c[ot])

            # ------------ stage 3: k projection (channel-major) ------------
            for ch in range(NCHUNKS):
                cs = slice(ch * NCH, (ch + 1) * NCH)
                for ot in range(4):
                    pk = ps_d.tile([128, NCH], F32, name="pk", tag="pz")
                    for kt in range(2):
                        nc.tensor.matmul(pk[:],
                                         wkv[kt][:, ot * 128:(ot + 1) * 128],
                                         y2T[kt][:, cs],
                                         start=(kt == 0), stop=(kt == 1))
                    nc.any.tensor_copy(kT[ot][:, cs], pk[:])

            # ------------ stage 4: q projection (channel-major) ------------
            for ch in range(NCHUNKS):
                cs = slice(ch * NCH, (ch + 1) * NCH)
                for ot in range(4):
                    pq = ps_d.tile([128, NCH], F32, name="pq", tag="pz")
                    for ct in range(4):
                        nc.tensor.matmul(pq[:],
                                         wq[ct][:, ot * 128:(ot + 1) * 128],
                                         xT[ct][:, cs],
                                         start=(ct == 0), stop=(ct == 3))
                    nc.any.tensor_copy(qT[ot][:, cs], pq[:])
            ps_d_cm.__exit__(None, None, None)

        # ------------ stage 5-7: v (window-major), attention, proj ------------
        # qT/kT/y2T columns are window-major: window w = wi*8+wj occupies
        # cols w*49:(w+1)*49. attT stays spatial-major (scatter on write).

        def win_view(t):
            return t.rearrange("p (a i b j) -> p a b i j", a=8, i=7, b=8, j=7)

        with tc.tile_pool(name="pool_att", bufs=1) as pool_att, \
             tc.tile_pool(name="ps_a", bufs=2, space="PSUM") as ps_a:
            attT = [pool_att.tile([128, N1], BF16, name=f"attT{t}", tag=f"attT{t}")
                    for t in range(4)]
            for wi in range(8):
                vw = pool_vw.tile([49, 8 * C1], BF16, name="vw", tag="vw")
                for wj in range(8):
                    wsl = slice((wi * 8 + wj) * 49, (wi * 8 + wj + 1) * 49)
                    pv = ps_a.tile([49, C1], F32, name="pv", tag="pv")
                    for kt in range(2):
                        nc.tensor.matmul(pv[:], y2T[kt][:, wsl],
                                         wkv[kt][:, C1:2 * C1],
                                         start=(kt == 0), stop=(kt == 1))
                    nc.scalar.copy(out=vw[:, wj * C1:(wj + 1) * C1], in_=pv[:])
                for h in range(8):
                    t, pb_ = h // 2, (h % 2) * 64
                    psl = slice(pb_, pb_ + 64)
                    S = ps_a.tile([49, 392], F32, name="S", tag="S")
                    for wj in range(8):
                        wsl = slice((wi * 8 + wj) * 49, (wi * 8 + wj + 1) * 49)
                        nc.tensor.matmul(S[:, wj * 49:(wj + 1) * 49],
                                         kT[t][psl, wsl],
                                         qT[t][psl, wsl],
                                         start=True, stop=True)
                    E = pool_tmp.tile([49, 392], BF16, name="E", tag="E", bufs=3)
                    nc.scalar.activation(out=E[:], in_=S[:], func=AF.Exp,
                                         scale=0.125)
                    SUMB = ps_a.tile([64, 392], F32, name="SUMB",
                                     tag="SUMB", bufs=1)
                    nc.tensor.matmul(SUMB[:], ones_s[:], E[:],
                                     start=True, stop=True)
                    RB = pool_tmp.tile([64, 392], F32, name="RB", tag="RB", bufs=3)
                    nc.vector.reciprocal(out=RB[:], in_=SUMB[:])
                    AV = ps_a.tile([64, 392], F32, name="AV", tag="AV")
                    for wj in range(8):
                        nc.tensor.matmul(
                            AV[:, wj * 49:(wj + 1) * 49],
                            vw[:, wj * C1 + h * 64:wj * C1 + (h + 1) * 64],
                            E[:, wj * 49:(wj + 1) * 49],
                            start=True, stop=True)
                    avv = AV.rearrange("p (b i j) -> p b i j", b=8, i=7, j=7)
                    rbv = RB.rearrange("p (b i j) -> p b i j", b=8, i=7, j=7)
                    nc.vector.tensor_mul(win_view(attT[t])[psl, wi],
                                         avv[:], rbv[:])

            # ------------ stage 7: output projection + int8 quantization ------------
            for nt in range(NT):
                nsz = min(128, N1 - nt * 128)
                ns = slice(nt * 128, nt * 128 + nsz)
                po = ps_a.tile([128, C1], F32, name="po", tag="pv")
                for ct in range(4):
                    nc.tensor.matmul(po[:nsz, :], attT[ct][:, ns], wp[ct][:],
                                     start=(ct == 0), stop=False)
                nc.tensor.matmul(po[:nsz, :], ones_r[:, :nsz], bp_sb[:],
                                 start=False, stop=True)
                # per-row absmax -> int8 scale; conversion rounds-to-nearest
                mx = pool_tmp.tile([128, 1], F32, name="mx", tag="mx", bufs=2)
                nc.vector.tensor_reduce(mx[:nsz, :], po[:nsz, :],
                                        axis=mybir.AxisListType.X,
                                        op=mybir.AluOpType.max,
                                        apply_absolute_value=True)
                nc.vector.tensor_scalar_max(mx[:nsz, :], mx[:nsz, :], 1e-30)
                rs = pool_tmp.tile([128, 1], F32, name="rs", tag="rs", bufs=2)
                nc.vector.reciprocal(out=rs[:nsz, :], in_=mx[:nsz, :])
                nc.vector.tensor_scalar_mul(rs[:nsz, :], rs[:nsz, :], 127.0)
                o_i8 = pool_tmp.tile([128, C1], I8, name="o_i8",
                                     tag="o_i8", bufs=2)
                nc.scalar.activation(out=o_i8[:nsz, :], in_=po[:nsz, :],
                                     func=AF.Identity, scale=rs[:nsz, :])
                nc.sync.dma_start(out=out[ns, :], in_=o_i8[:nsz, :])
                nc.sync.dma_start(out=scl[:nsz, nt:nt + 1], in_=mx[:nsz, :])


def _get_nc(w):
    rebuild = True
    if "nc" in _cache:
        old = _cache["w"]
        rebuild = not all(np.array_equal(old[k], w[k]) for k in old)
    if rebuild:
        _cache["nc"] = _build_nc(w)
        _cache["w"] = w
    return _cache["nc"]


def _quant_rows(a, pow2=1.0):
    """Per-row symmetric int8: returns (int8 array, dequant scale per row)."""
    s = np.abs(a).max(-1, keepdims=True)
    np.maximum(s, 1e-30, out=s)
    t = a * (127.0 / s)
    t += 128.5
    u = t.astype(np.uint8)
    np.bitwise_xor(u, 128, out=u)
    return u.view(np.int8), (s * (1.0 / (127.0 * pow2))).astype(np.float32)


def _executor():
    if "pool" not in _cache:
        from concurrent.futures import ThreadPoolExecutor
        _cache["pool"] = ThreadPoolExecutor(max_workers=4)
    return _cache["pool"]


def kernel(**inputs):
    import ml_dtypes
    bf16 = ml_dtypes.bfloat16
    f32 = np.float32

    x = np.asarray(inputs["x"], dtype=f32)
    y = np.asarray(inputs["y"], dtype=f32)
    Wq = np.asarray(inputs["Wq"], dtype=f32)
    Wkv = np.asarray(inputs["Wkv"], dtype=f32)
    Wproj = np.asarray(inputs["Wproj"], dtype=f32)
    bproj = np.asarray(inputs["bproj"], dtype=f32)
    bsr_np = np.asarray(inputs["bsr"], dtype=f32)
    Wsr = np.asarray(inputs["Wsr"], dtype=f32)
    gn = np.asarray(inputs["gn"], dtype=f32)
    bn = np.asarray(inputs["bn"], dtype=f32)

    w = {
        "WqT": np.ascontiguousarray(Wq.T).astype(bf16),
        "WsrT": np.ascontiguousarray(Wsr.T).astype(bf16),
        "WkvT": np.ascontiguousarray(Wkv.T).astype(bf16),
        "WpT": np.ascontiguousarray(Wproj.T).astype(bf16),
        "bsr": bsr_np,
        "gnr": np.ascontiguousarray(gn.reshape(2, 128)).astype(f32),
        "bnc": bn,
        "bp": np.ascontiguousarray(bproj.reshape(1, C1)).astype(bf16),
        "eye": np.eye(HP, dtype=bf16),
    }

    # x: per-row int8 (natural layout; device transposes + window-majors)
    # y: 2x2 sum-pool on host, per-row int8; /4 (pool mean) folded into scales
    def quant_x(bs):
        return _quant_rows(x.reshape(B, N1, C1)[bs])

    def quant_y(bs):
        yb = y.reshape(B, H2, HP, 2, C2)[bs]
        s1 = yb.sum(3)
        n = s1.shape[0]
        ysum = s1.reshape(n, HP, 2, HP, C2).sum(2).reshape(n, N1, C2)
        return _quant_rows(ysum, pow2=4.0)

    ex = _executor()
    h = slice(0, B // 2)
    t = slice(B // 2, B)
    futs = [ex.submit(quant_x, h), ex.submit(quant_x, t),
            ex.submit(quant_y, h), ex.submit(quant_y, t)]
    (xi8a, xsca), (xi8b, xscb), (yi8a, ysca), (yi8b, yscb) = \
        [f.result() for f in futs]
    xi8 = [xi8a[b] for b in range(4)] + [xi8b[b] for b in range(4)]
    xsc = [xsca[b] for b in range(4)] + [xscb[b] for b in range(4)]
    yi8 = [yi8a[b] for b in range(4)] + [yi8b[b] for b in range(4)]
    ysc = [ysca[b] for b in range(4)] + [yscb[b] for b in range(4)]

    nc = _get_nc(w)
    in_maps = [{"xN": xi8[b], "xs": xsc[b], "ypN": yi8[b], "yps": ysc[b]}
               for b in range(B)]
    from concourse.bass_utils import run_bass_kernel_spmd
    res = run_bass_kernel_spmd(nc, in_maps, core_ids=list(range(B)),
                               **_cache.get("run_opts", {}))
    _cache["last_res"] = res

    # dequantize: row n = nt*128 + p  ->  scale scl[p, nt] / 127
    i8 = np.stack([r["out"] for r in res.results], axis=0)      # (B, N1, C1) int8
    sc = np.stack([r["scl"] for r in res.results], axis=0)      # (B, 128, NT) f32
    srow = sc.transpose(0, 2, 1).reshape(B, NT * 128)[:, :N1] * (1.0 / 127.0)
    return i8 * srow[:, :, None]


# revision 16
# speedup vs baseline: 3.8129x; 1.0492x over previous
"""Trainium2 Bass kernel for windowed cross-attention (nn_CrossAttention_37056977830404).

Sharding: data-parallel over batch B=8 across the 8 NeuronCores (one batch
element per core). The call is transfer-bound over the axon tunnel
(~40 MB/s in, ~28 MB/s out), so the design minimizes bytes moved:
  - weights are baked into the NEFF as Const tensors (loaded once),
  - y is 2x2 sum-pooled on the host (4x reduction),
  - x and pooled-y are sent as int8 with per-row scales (2x vs bf16),
  - the output is int8 with per-row scales (4x smaller than f32, and the
    donated zero output buffers the PJRT path uploads shrink the same way).

Per-core pipeline (all shapes hardcoded):
  stage 0: xN [3136,512] i8, ypN [3136,256] i8 (natural layout) are
  dequantized to bf16 (per-row scales) and transposed on the tensor engine
  (identity matmuls, 56-row chunks) into window-major channel-major SBUF
  tiles: col n' = (wi*8+wj)*49 + i*7 + j.
  z = yp @ Wsr.T + bsr  (bf16 matmul, fp32 psum)     [sr conv; /4 in scales]
  LN over channels (cross-partition ones-matmul sums) + gelu -> y2T bf16
  kT = (y2 @ Wkv_k.T).T     [channel-major, bf16]
  v_w = y2 @ Wkv_v.T        [window-major via windowed stationary APs, bf16]
  qT = (x @ Wq.T).T         [channel-major, bf16]
  per (head, window-row): S^T = k_w^T q_w ; E = exp(S^T/8) ; sums via
  ones-matmul broadcast ; AV = v_w^T E ; attT = AV * recip(sum)  [bf16]
  out = attT.T @ Wproj.T + bproj  (bf16 matmuls), then per-row int8
  quantization: scl = absmax(row), out_i8 = round(out * 127/scl).
"""
import os
import sys

sys.path.insert(0, '/opt/trn_rl_repo')
os.environ.setdefault("JAX_COMPILATION_CACHE_DIR", "/tmp/jax_ccache")
os.environ.setdefault("JAX_PERSISTENT_CACHE_MIN_COMPILE_TIME_SECS", "0")
os.environ.setdefault("JAX_PERSISTENT_CACHE_MIN_ENTRY_SIZE_BYTES", "0")
import numpy as np

try:  # env vars above are too late if jax was imported first; force via config
    import jax as _jax
    _jax.config.update("jax_compilation_cache_dir", "/tmp/jax_ccache")
    _jax.config.update("jax_persistent_cache_min_compile_time_secs", 0)
    _jax.config.update("jax_persistent_cache_min_entry_size_bytes", 0)
except Exception:
    pass

B = 8
C1 = 512
N1 = 3136
NH = 8
HD = 64
WS = 7
C2 = 256
H2 = W2 = 112
HP = WP = 56
NCH = 392      # dense matmul n-chunk (free dim) = one window-row
NCHUNKS = 8    # 3136 / 392
NT = 25        # output row tiles (24x128 + 64)
EPS = 1e-5

_cache = {}


def _build_nc(w):
    import concourse.bacc as bacc
    import concourse.tile as tile
    from concourse import mybir

    F32 = mybir.dt.float32
    BF16 = mybir.dt.bfloat16
    I8 = mybir.dt.int8

    nc = bacc.Bacc()

    # ---------------- DRAM I/O ----------------
    # int8 data rows, with the f32 per-row dequant scales packed into extra
    # rows at the bottom (fewer tensors -> fewer per-transfer latencies)
    xN = nc.dram_tensor("xN", [N1 + 25, C1], I8, kind="ExternalInput")
    ypN = nc.dram_tensor("ypN", [N1 + 49, C2], I8, kind="ExternalInput")
    out = nc.dram_tensor("out", [N1 + 25, C1], I8, kind="ExternalOutput")
    # weights baked into the NEFF (DMA'd to HBM once at model load)
    consts = {
        "WqT": nc.inline_tensor(w["WqT"], name="cWqT"),
        "WsrT": nc.inline_tensor(w["WsrT"], name="cWsrT"),
        "WkvT": nc.inline_tensor(w["WkvT"], name="cWkvT"),
        "WpT": nc.inline_tensor(w["WpT"], name="cWpT"),
        "bsr": nc.inline_tensor(w["bsr"], name="cbsr"),
        "gnr": nc.inline_tensor(w["gnr"], name="cgnr"),
        "bnc": nc.inline_tensor(w["bnc"], name="cbnc"),
        "bp": nc.inline_tensor(w["bp"], name="cbp"),
        "eye": nc.inline_tensor(w["eye"], name="ceye"),
    }

    with tile.TileContext(nc) as tc:
        _emit(nc, tc, mybir, F32, BF16, I8, xN, ypN, consts, out)
    nc.finalize()
    return nc


def _emit(nc, tc, mybir, F32, BF16, I8, xN, ypN, consts, out):
    from contextlib import ExitStack

    F32R = mybir.dt.float32r
    AF = mybir.ActivationFunctionType
    WqT, WsrT, WkvT, WpT = (consts["WqT"], consts["WsrT"], consts["WkvT"],
                            consts["WpT"])
    bsr, gnr, bnc, bp = consts["bsr"], consts["gnr"], consts["bnc"], consts["bp"]

    with ExitStack() as ctx:
        pool_w = ctx.enter_context(tc.tile_pool(name="pool_w", bufs=1))
        pool_big = ctx.enter_context(tc.tile_pool(name="pool_big", bufs=1))
        pool_vw = ctx.enter_context(tc.tile_pool(name="pool_vw", bufs=2))
        pool_tmp = ctx.enter_context(tc.tile_pool(name="pool_tmp", bufs=2))

        # ---------------- weights / constants to SBUF ----------------
        wq, wp, wsr, wkv = [], [], [], []
        for ct in range(4):
            wq_t = pool_w.tile([128, C1], BF16, name=f"wq{ct}", tag=f"wq{ct}")
            nc.sync.dma_start(out=wq_t, in_=WqT[ct * 128:(ct + 1) * 128, :])
            wq.append(wq_t)
            wp_t = pool_w.tile([128, C1], BF16, name=f"wp{ct}", tag=f"wp{ct}")
            nc.sync.dma_start(out=wp_t, in_=WpT[ct * 128:(ct + 1) * 128, :])
            wp.append(wp_t)
        for kt in range(2):
            wsr_t = pool_w.tile([128, C2], BF16, name=f"wsr{kt}", tag=f"wsr{kt}")
            nc.sync.dma_start(out=wsr_t, in_=WsrT[kt * 128:(kt + 1) * 128, :])
            wsr.append(wsr_t)
            wkv_t = pool_w.tile([128, 2 * C1], BF16, name=f"wkv{kt}", tag=f"wkv{kt}")
            nc.sync.dma_start(out=wkv_t, in_=WkvT[kt * 128:(kt + 1) * 128, :])
            wkv.append(wkv_t)
        bsr_c, bn_c, gn_r = [], [], []
        for ot in range(2):
            b1 = pool_w.tile([128, 1], F32, name=f"bsr{ot}", tag=f"bsr{ot}")
            nc.sync.dma_start(out=b1, in_=bsr[ot * 128:(ot + 1) * 128].unsqueeze(1))
            bsr_c.append(b1)
            b2 = pool_w.tile([128, 1], F32, name=f"bn{ot}", tag=f"bn{ot}")
            nc.sync.dma_start(out=b2, in_=bnc[ot * 128:(ot + 1) * 128].unsqueeze(1))
            bn_c.append(b2)
            g0 = pool_w.tile([1, 128], F32, name=f"gnrf{ot}", tag=f"gnrf{ot}")
            nc.sync.dma_start(out=g0, in_=gnr[ot:ot + 1, :])
            g1 = pool_w.tile([1, 128], F32R, name=f"gnr{ot}", tag=f"gnr{ot}")
            nc.vector.tensor_copy(g1[:], g0[:])
            gn_r.append(g1)
        bp_sb = pool_w.tile([1, C1], BF16, name="bp_sb", tag="bp_sb")
        nc.sync.dma_start(out=bp_sb, in_=bp.ap())
        eye_sb = pool_w.tile([HP, HP], BF16, name="eye_sb", tag="eye_sb")
        nc.sync.dma_start(out=eye_sb, in_=consts["eye"].ap())
        # per-row dequant scales from the packed f32 rows (bitcast views),
        # transposed load: xs_all[p, r] = xs[r*56+p]
        xflat = xN.bitcast(F32)[N1:N1 + 25, :].rearrange("a b -> (a b)")
        xs_all = pool_w.tile([HP, HP], F32, name="xs_all", tag="xs_all")
        nc.sync.dma_start(out=xs_all,
                          in_=xflat[:N1].rearrange("(r p) -> p r", r=HP, p=HP))
        yflat = ypN.bitcast(F32)[N1:N1 + 49, :].rearrange("a b -> (a b)")
        ys_all = pool_w.tile([HP, HP], F32, name="ys_all", tag="ys_all")
        nc.sync.dma_start(out=ys_all,
                          in_=yflat[:N1].rearrange("(r p) -> p r", r=HP, p=HP))

        ones_f = pool_w.tile([128, 1], F32, name="ones_f", tag="ones_f")
        nc.vector.memset(ones_f, 1.0)
        ones_c = pool_w.tile([128, 1], F32R, name="ones_c", tag="ones_c")
        nc.vector.tensor_copy(ones_c[:], ones_f[:])
        ones_rf = pool_w.tile([1, 128], F32, name="ones_rf", tag="ones_rf")
        nc.vector.memset(ones_rf, 1.0)
        ones_r = pool_w.tile([1, 128], BF16, name="ones_r", tag="ones_r")
        nc.vector.tensor_copy(ones_r[:], ones_rf[:])
        ones_s = pool_w.tile([49, 64], BF16, name="ones_s", tag="ones_s")
        nc.vector.memset(ones_s, 1.0)
        eps_sb = pool_w.tile([1, 1], F32, name="eps_sb", tag="eps_sb")
        nc.vector.memset(eps_sb, EPS)

        # ---------------- persistent activations ----------------
        xT = [pool_big.tile([128, N1], BF16, name=f"xT{t}", tag=f"xT{t}")
              for t in range(4)]
        y2T = [pool_big.tile([128, N1], BF16, name=f"y2T{k}", tag=f"y2T{k}")
               for k in range(2)]
        kT = [pool_big.tile([128, N1], BF16, name=f"kT{t}", tag=f"kT{t}")
              for t in range(4)]
        qT = [pool_big.tile([128, N1], BF16, name=f"qT{t}", tag=f"qT{t}")
              for t in range(4)]

        def wdest(tile_, r):
            # window-major scatter view for spatial row r: [p, wj 8, j 7]
            # target col = (wi*8+wj)*49 + i*7 + j
            wi, i = r // WS, r % WS
            v = tile_.rearrange("p (a b i j) -> p a b i j", a=8, b=8, i=7, j=7)
            return v[:, wi, :, i]

        with tc.tile_pool(name="pool_yp", bufs=1) as pool_yp:
            ypT = [pool_yp.tile([128, N1], BF16, name=f"ypT{k}", tag=f"ypT{k}")
                   for k in range(2)]

            # ------------ stage 0: dequant + transpose to window-major ------------
            with tc.tile_pool(name="ps_t", bufs=1, space="PSUM") as ps_t:
                for r in range(HP):
                    rs_ = slice(r * HP, (r + 1) * HP)
                    xi = pool_tmp.tile([HP, C1], I8, name="xi", tag="xi", bufs=3)
                    nc.sync.dma_start(out=xi, in_=xN[rs_, :])
                    xb = pool_tmp.tile([HP, C1], BF16, name="xb", tag="xb", bufs=3)
                    nc.scalar.activation(out=xb[:], in_=xi[:], func=AF.Identity,
                                         scale=xs_all[:, r:r + 1])
                    yi = pool_tmp.tile([HP, C2], I8, name="yi", tag="yi", bufs=3)
                    nc.sync.dma_start(out=yi, in_=ypN[rs_, :])
                    yb = pool_tmp.tile([HP, C2], BF16, name="yb", tag="yb", bufs=3)
                    nc.scalar.activation(out=yb[:], in_=yi[:], func=AF.Identity,
                                         scale=ys_all[:, r:r + 1])
                    for cb in range(4):
                        pt = ps_t.tile([128, HP], F32, name="pt", tag="pt", bufs=6)
                        nc.tensor.matmul(pt[:], xb[:, cb * 128:(cb + 1) * 128],
                                         eye_sb[:], start=True, stop=True)
                        nc.vector.tensor_copy(
                            wdest(xT[cb], r),
                            pt.rearrange("p (b j) -> p b j", b=8, j=7))
                    for cb in range(2):
                        pt = ps_t.tile([128, HP], F32, name="pt2", tag="pt", bufs=6)
                        nc.tensor.matmul(pt[:], yb[:, cb * 128:(cb + 1) * 128],
                                         eye_sb[:], start=True, stop=True)
                        nc.vector.tensor_copy(
                            wdest(ypT[cb], r),
                            pt.rearrange("p (b j) -> p b j", b=8, j=7))

            ps_d_cm = tc.tile_pool(name="ps_d", bufs=2, space="PSUM")
            ps_d = ps_d_cm.__enter__()
            # ------------ stage 2: sr conv + LN + gelu ------------
            for ch in range(NCHUNKS):
                cs = slice(ch * NCH, (ch + 1) * NCH)
                zsb = []
                for ot in range(2):
                    pz = ps_d.tile([128, NCH], F32, name="pz", tag="pz")
                    for kt in range(2):
                        nc.tensor.matmul(pz[:], wsr[kt][:, ot * 128:(ot + 1) * 128],
                                         ypT[kt][:, cs],
                                         start=(kt == 0), stop=(kt == 1))
                    z_t = pool_tmp.tile([128, NCH], F32R, name="z_t",
                                        tag="zsb", bufs=4)
                    nc.scalar.activation(out=z_t[:], in_=pz[:], func=AF.Identity,
                                         bias=bsr_c[ot])
                    zsb.append(z_t)
                pst_s = ps_d.tile([1, NCH], F32, name="pst_s", tag="pst_s", bufs=1)
                pst_q = ps_d.tile([1, NCH], F32, name="pst_q", tag="pst_q", bufs=1)
                for ot in range(2):
                    nc.tensor.matmul(pst_s[:], ones_c[:], zsb[ot][:],
                                     start=(ot == 0), stop=(ot == 1))
                for ot in range(2):
                    zq = pool_tmp.tile([128, NCH], F32R, name="zq", tag="zq", bufs=2)
                    nc.scalar.activation(out=zq[:], in_=zsb[ot][:], func=AF.Square)
                    nc.tensor.matmul(pst_q[:], ones_c[:], zq[:],
                                     start=(ot == 0), stop=(ot == 1))
                m_sb = pool_tmp.tile([1, NCH], F32, name="m_sb", tag="m_sb", bufs=1)
                nc.vector.tensor_scalar_mul(m_sb[:], pst_s[:], 1.0 / C2)
                q_sb = pool_tmp.tile([1, NCH], F32, name="q_sb", tag="q_sb", bufs=1)
                nc.vector.tensor_scalar_mul(q_sb[:], pst_q[:], 1.0 / C2)
                var_sb = pool_tmp.tile([1, NCH], F32, name="var_sb",
                                       tag="var_sb", bufs=1)
                nc.gpsimd.tensor_tensor(var_sb[:], m_sb[:], m_sb[:],
                                        op=mybir.AluOpType.mult)
                nc.gpsimd.tensor_tensor(var_sb[:], q_sb[:], var_sb[:],
                                        op=mybir.AluOpType.subtract)
                sd_sb = pool_tmp.tile([1, NCH], F32, name="sd_sb",
                                      tag="sd_sb", bufs=1)
                nc.scalar.activation(out=sd_sb[:], in_=var_sb[:], func=AF.Sqrt,
                                     bias=eps_sb[:])
                r_sb = pool_tmp.tile([1, NCH], F32R, name="r_sb", tag="r_sb", bufs=1)
                with nc.allow_low_precision(reason="f32r rstd feeds f32r matmul"):
                    nc.vector.reciprocal(out=r_sb[:], in_=sd_sb[:])
                nb_sb = pool_tmp.tile([1, NCH], F32R, name="nb_sb",
                                      tag="nb_sb", bufs=1)
                nc.gpsimd.tensor_tensor(nb_sb[:], m_sb[:], r_sb[:],
                                        op=mybir.AluOpType.mult)
                nc.gpsimd.tensor_scalar_mul(nb_sb[:], nb_sb[:], -1.0)
                for ot in range(2):
                    pa = ps_d.tile([128, NCH], F32, name="pa", tag="pa")
                    nc.tensor.matmul(pa[:], gn_r[ot][:], r_sb[:],
                                     start=True, stop=True)
                    pb = ps_d.tile([128, NCH], F32, name="pb", tag="pb")
                    nc.tensor.matmul(pb[:], gn_r[ot][:], nb_sb[:],
                                     start=True, stop=True)
                    t1 = pool_tmp.tile([128, NCH], F32, name="t1", tag="t1", bufs=2)
                    nc.vector.tensor_mul(t1[:], zsb[ot][:], pa[:])
                    nc.vector.tensor_add(t1[:], t1[:], pb[:])
                    nc.scalar.activation(out=y2T[ot][:, cs], in_=t1[:],
                                         func=AF.Gelu, bias=bn_c[ot])

            # ------------ stage 3: k projection (channel-major) ------------
            for ch in range(NCHUNKS):
                cs = slice(ch * NCH, (ch + 1) * NCH)
                for ot in range(4):
                    pk = ps_d.tile([128, NCH], F32, name="pk", tag="pz")
                    for kt in range(2):
                        nc.tensor.matmul(pk[:],
                                         wkv[kt][:, ot * 128:(ot + 1) * 128],
                                         y2T[kt][:, cs],
                                         start=(kt == 0), stop=(kt == 1))
                    nc.any.tensor_copy(kT[ot][:, cs], pk[:])

            # ------------ stage 4: q projection (channel-major) ------------
            for ch in range(NCHUNKS):
                cs = slice(ch * NCH, (ch + 1) * NCH)
                for ot in range(4):
                    pq = ps_d.tile([128, NCH], F32, name="pq", tag="pz")
                    for ct in range(4):
                        nc.tensor.matmul(pq[:],
                                         wq[ct][:, ot * 128:(ot + 1) * 128],
                                         xT[ct][:, cs],
                                         start=(ct == 0), stop=(ct == 3))
                    nc.any.tensor_copy(qT[ot][:, cs], pq[:])
            ps_d_cm.__exit__(None, None, None)

        # ------------ stage 5-7: v (window-major), attention, proj ------------
        # qT/kT/y2T columns are window-major: window w = wi*8+wj occupies
        # cols w*49:(w+1)*49. attT stays spatial-major (scatter on write).

        def win_view(t):
            return t.rearrange("p (a i b j) -> p a b i j", a=8, i=7, b=8, j=7)

        with tc.tile_pool(name="pool_att", bufs=1) as pool_att, \
             tc.tile_pool(name="ps_a", bufs=2, space="PSUM") as ps_a:
            attT = [pool_att.tile([128, N1], BF16, name=f"attT{t}", tag=f"attT{t}")
                    for t in range(4)]
            for wi in range(8):
                vw = pool_vw.tile([49, 8 * C1], BF16, name="vw", tag="vw")
                for wj in range(8):
                    wsl = slice((wi * 8 + wj) * 49, (wi * 8 + wj + 1) * 49)
                    pv = ps_a.tile([49, C1], F32, name="pv", tag="pv")
                    for kt in range(2):
                        nc.tensor.matmul(pv[:], y2T[kt][:, wsl],
                                         wkv[kt][:, C1:2 * C1],
                                         start=(kt == 0), stop=(kt == 1))
                    nc.scalar.copy(out=vw[:, wj * C1:(wj + 1) * C1], in_=pv[:])
                for h in range(8):
                    t, pb_ = h // 2, (h % 2) * 64
                    psl = slice(pb_, pb_ + 64)
                    S = ps_a.tile([49, 392], F32, name="S", tag="S")
                    for wj in range(8):
                        wsl = slice((wi * 8 + wj) * 49, (wi * 8 + wj + 1) * 49)
                        nc.tensor.matmul(S[:, wj * 49:(wj + 1) * 49],
                                         kT[t][psl, wsl],
                                         qT[t][psl, wsl],
                                         start=True, stop=True)
                    E = pool_tmp.tile([49, 392], BF16, name="E", tag="E", bufs=3)
                    nc.scalar.activation(out=E[:], in_=S[:], func=AF.Exp,
                                         scale=0.125)
                    SUMB = ps_a.tile([64, 392], F32, name="SUMB",
                                     tag="SUMB", bufs=1)
                    nc.tensor.matmul(SUMB[:], ones_s[:], E[:],
                                     start=True, stop=True)
                    RB = pool_tmp.tile([64, 392], F32, name="RB", tag="RB", bufs=3)
                    nc.vector.reciprocal(out=RB[:], in_=SUMB[:])
                    AV = ps_a.tile([64, 392], F32, name="AV", tag="AV")
                    for wj in range(8):
                        nc.tensor.matmul(
                            AV[:, wj * 49:(wj + 1) * 49],
                            vw[:, wj * C1 + h * 64:wj * C1 + (h + 1) * 64],
                            E[:, wj * 49:(wj + 1) * 49],
                            start=True, stop=True)
                    avv = AV.rearrange("p (b i j) -> p b i j", b=8, i=7, j=7)
                    rbv = RB.rearrange("p (b i j) -> p b i j", b=8, i=7, j=7)
                    nc.vector.tensor_mul(win_view(attT[t])[psl, wi],
                                         avv[:], rbv[:])

            # ------------ stage 7: output projection + int8 quantization ------------
            for nt in range(NT):
                nsz = min(128, N1 - nt * 128)
                ns = slice(nt * 128, nt * 128 + nsz)
                po = ps_a.tile([128, C1], F32, name="po", tag="pv")
                for ct in range(4):
                    nc.tensor.matmul(po[:nsz, :], attT[ct][:, ns], wp[ct][:],
                                     start=(ct == 0), stop=False)
                nc.tensor.matmul(po[:nsz, :], ones_r[:, :nsz], bp_sb[:],
                                 start=False, stop=True)
                # per-row absmax -> int8 scale; conversion rounds-to-nearest
                mx = pool_tmp.tile([128, 1], F32, name="mx", tag="mx", bufs=2)
                nc.vector.tensor_reduce(mx[:nsz, :], po[:nsz, :],
                                        axis=mybir.AxisListType.X,
                                        op=mybir.AluOpType.max,
                                        apply_absolute_value=True)
                nc.vector.tensor_scalar_max(mx[:nsz, :], mx[:nsz, :], 1e-30)
                rs = pool_tmp.tile([128, 1], F32, name="rs", tag="rs", bufs=2)
                nc.vector.reciprocal(out=rs[:nsz, :], in_=mx[:nsz, :])
                nc.vector.tensor_scalar_mul(rs[:nsz, :], rs[:nsz, :], 127.0)
                o_i8 = pool_tmp.tile([128, C1], I8, name="o_i8",
                                     tag="o_i8", bufs=2)
                nc.scalar.activation(out=o_i8[:nsz, :], in_=po[:nsz, :],
                                     func=AF.Identity, scale=rs[:nsz, :])
                nc.sync.dma_start(out=out[ns, :], in_=o_i8[:nsz, :])
                nc.sync.dma_start(out=out.bitcast(F32)[N1 + nt:N1 + nt + 1, :nsz],
                                  in_=mx[:nsz, :])


def _get_nc(w):
    rebuild = True
    if "nc" in _cache:
        old = _cache["w"]
        rebuild = not all(np.array_equal(old[k], w[k]) for k in old)
    if rebuild:
        _cache["nc"] = _build_nc(w)
        _cache["w"] = w
    return _cache["nc"]


def _quant_rows(a, pow2=1.0):
    """Per-row symmetric int8: returns (int8 array, dequant scale per row)."""
    s = np.abs(a).max(-1, keepdims=True)
    np.maximum(s, 1e-30, out=s)
    t = a * (127.0 / s)
    t += 128.5
    u = t.astype(np.uint8)
    np.bitwise_xor(u, 128, out=u)
    return u.view(np.int8), (s * (1.0 / (127.0 * pow2))).astype(np.float32)


def _executor():
    if "pool" not in _cache:
        from concurrent.futures import ThreadPoolExecutor
        _cache["pool"] = ThreadPoolExecutor(max_workers=4)
    return _cache["pool"]


def kernel(**inputs):
    import ml_dtypes
    bf16 = ml_dtypes.bfloat16
    f32 = np.float32

    x = np.asarray(inputs["x"], dtype=f32)
    y = np.asarray(inputs["y"], dtype=f32)
    Wq = np.asarray(inputs["Wq"], dtype=f32)
    Wkv = np.asarray(inputs["Wkv"], dtype=f32)
    Wproj = np.asarray(inputs["Wproj"], dtype=f32)
    bproj = np.asarray(inputs["bproj"], dtype=f32)
    bsr_np = np.asarray(inputs["bsr"], dtype=f32)
    Wsr = np.asarray(inputs["Wsr"], dtype=f32)
    gn = np.asarray(inputs["gn"], dtype=f32)
    bn = np.asarray(inputs["bn"], dtype=f32)

    w = {
        "WqT": np.ascontiguousarray(Wq.T).astype(bf16),
        "WsrT": np.ascontiguousarray(Wsr.T).astype(bf16),
        "WkvT": np.ascontiguousarray(Wkv.T).astype(bf16),
        "WpT": np.ascontiguousarray(Wproj.T).astype(bf16),
        "bsr": bsr_np,
        "gnr": np.ascontiguousarray(gn.reshape(2, 128)).astype(f32),
        "bnc": bn,
        "bp": np.ascontiguousarray(bproj.reshape(1, C1)).astype(bf16),
        "eye": np.eye(HP, dtype=bf16),
    }

    # x: per-row int8 (natural layout; device transposes + window-majors)
    # y: 2x2 sum-pool on host, per-row int8; /4 (pool mean) folded into scales
    # The f32 dequant scales are packed into extra int8 rows of each buffer.
    xbuf = np.empty((B, N1 + 25, C1), np.int8)
    ybuf = np.empty((B, N1 + 49, C2), np.int8)

    def pack(buf, bs, i8, sc):
        np.copyto(buf[bs, :N1, :], i8)
        for k, b in enumerate(range(bs.start, bs.stop)):
            buf[b, N1:].reshape(-1).view(np.float32)[:N1] = sc[k, :, 0]

    def quant_x(bs):
        i8, sc = _quant_rows(x.reshape(B, N1, C1)[bs])
        pack(xbuf, bs, i8, sc)

    def quant_y(bs):
        yb = y.reshape(B, H2, HP, 2, C2)[bs]
        s1 = yb.sum(3)
        n = s1.shape[0]
        ysum = s1.reshape(n, HP, 2, HP, C2).sum(2).reshape(n, N1, C2)
        i8, sc = _quant_rows(ysum, pow2=4.0)
        pack(ybuf, bs, i8, sc)

    ex = _executor()
    h = slice(0, B // 2)
    t = slice(B // 2, B)
    futs = [ex.submit(quant_x, h), ex.submit(quant_x, t),
            ex.submit(quant_y, h), ex.submit(quant_y, t)]
    for f in futs:
        f.result()

    nc = _get_nc(w)
    in_maps = [{"xN": xbuf[b], "ypN": ybuf[b]} for b in range(B)]
    from concourse.bass_utils import run_bass_kernel_spmd
    res = run_bass_kernel_spmd(nc, in_maps, core_ids=list(range(B)),
                               **_cache.get("run_opts", {}))
    _cache["last_res"] = res

    # dequantize: scale for out row n is packed f32 element n of the tail rows
    outs = np.stack([r["out"] for r in res.results], axis=0)  # (B, N1+25, C1) i8
    i8 = outs[:, :N1, :]
    srow = np.ascontiguousarray(outs[:, N1:, :]).reshape(B, -1) \
             .view(np.float32)[:, :N1] * (1.0 / 127.0)
    return i8 * srow[:, :, None]


# revision 17
# speedup vs baseline: 3.8132x; 1.0001x over previous
"""Trainium2 Bass kernel for windowed cross-attention (nn_CrossAttention_37056977830404).

Sharding: data-parallel over batch B=8 across the 8 NeuronCores (one batch
element per core). The call is transfer-bound over the axon tunnel
(~40 MB/s in, ~28 MB/s out), so the design minimizes bytes moved:
  - weights are baked into the NEFF as Const tensors (loaded once),
  - y is 2x2 sum-pooled on the host (4x reduction),
  - x and pooled-y are sent as int8 with per-row scales (2x vs bf16),
  - the output is int8 with per-row scales (4x smaller than f32, and the
    donated zero output buffers the PJRT path uploads shrink the same way).

Per-core pipeline (all shapes hardcoded):
  stage 0: xN [3136,512] i8, ypN [3136,256] i8 (natural layout) are
  dequantized to bf16 (per-row scales) and transposed on the tensor engine
  (identity matmuls, 56-row chunks) into window-major channel-major SBUF
  tiles: col n' = (wi*8+wj)*49 + i*7 + j.
  z = yp @ Wsr.T + bsr  (bf16 matmul, fp32 psum)     [sr conv; /4 in scales]
  LN over channels (cross-partition ones-matmul sums) + gelu -> y2T bf16
  kT = (y2 @ Wkv_k.T).T     [channel-major, bf16]
  v_w = y2 @ Wkv_v.T        [window-major via windowed stationary APs, bf16]
  qT = (x @ Wq.T).T         [channel-major, bf16]
  per (head, window-row): S^T = k_w^T q_w ; E = exp(S^T/8) ; sums via
  ones-matmul broadcast ; AV = v_w^T E ; attT = AV * recip(sum)  [bf16]
  out = attT.T @ Wproj.T + bproj  (bf16 matmuls), then per-row int8
  quantization: scl = absmax(row), out_i8 = round(out * 127/scl).
"""
import os
import sys

sys.path.insert(0, '/opt/trn_rl_repo')
os.environ.setdefault("JAX_COMPILATION_CACHE_DIR", "/tmp/jax_ccache")
os.environ.setdefault("JAX_PERSISTENT_CACHE_MIN_COMPILE_TIME_SECS", "0")
os.environ.setdefault("JAX_PERSISTENT_CACHE_MIN_ENTRY_SIZE_BYTES", "0")
import numpy as np

try:  # env vars above are too late if jax was imported first; force via config
    import jax as _jax
    _jax.config.update("jax_compilation_cache_dir", "/tmp/jax_ccache")
    _jax.config.update("jax_persistent_cache_min_compile_time_secs", 0)
    _jax.config.update("jax_persistent_cache_min_entry_size_bytes", 0)
except Exception:
    pass

B = 8
C1 = 512
N1 = 3136
NH = 8
HD = 64
WS = 7
C2 = 256
H2 = W2 = 112
HP = WP = 56
NCH = 392      # dense matmul n-chunk (free dim) = one window-row
NCHUNKS = 8    # 3136 / 392
NT = 25        # output row tiles (24x128 + 64)
EPS = 1e-5

_cache = {}


def _build_nc(w):
    import concourse.bacc as bacc
    import concourse.tile as tile
    from concourse import mybir

    F32 = mybir.dt.float32
    BF16 = mybir.dt.bfloat16
    I8 = mybir.dt.int8

    nc = bacc.Bacc()

    # ---------------- DRAM I/O ----------------
    # One packed int8 input: x rows [0,3161) (3136 data + 25 f32-scale rows),
    # then the pooled-y region (3136 x 256 data + 49 x 256 scale bytes) packed
    # two 256B rows per 512B row, 1593 rows, 256B tail pad. One tensor ->
    # one transfer latency + one concat.
    xyN = nc.dram_tensor("xyN", [N1 + 25 + 1593, C1], I8, kind="ExternalInput")
    out = nc.dram_tensor("out", [N1 + 25, C1], I8, kind="ExternalOutput")
    # weights baked into the NEFF (DMA'd to HBM once at model load)
    consts = {
        "WqT": nc.inline_tensor(w["WqT"], name="cWqT"),
        "WsrT": nc.inline_tensor(w["WsrT"], name="cWsrT"),
        "WkvT": nc.inline_tensor(w["WkvT"], name="cWkvT"),
        "WpT": nc.inline_tensor(w["WpT"], name="cWpT"),
        "bsr": nc.inline_tensor(w["bsr"], name="cbsr"),
        "gnr": nc.inline_tensor(w["gnr"], name="cgnr"),
        "bnc": nc.inline_tensor(w["bnc"], name="cbnc"),
        "bp": nc.inline_tensor(w["bp"], name="cbp"),
        "eye": nc.inline_tensor(w["eye"], name="ceye"),
    }

    with tile.TileContext(nc) as tc:
        _emit(nc, tc, mybir, F32, BF16, I8, xyN, consts, out)
    nc.finalize()
    return nc


def _emit(nc, tc, mybir, F32, BF16, I8, xyN, consts, out):
    xN = xyN  # x region: rows [0, N1+25)
    ypN = xyN.ap().rearrange("a (h c) -> (a h) c", h=2, c=C2)[2 * (N1 + 25):]
    from contextlib import ExitStack

    F32R = mybir.dt.float32r
    AF = mybir.ActivationFunctionType
    WqT, WsrT, WkvT, WpT = (consts["WqT"], consts["WsrT"], consts["WkvT"],
                            consts["WpT"])
    bsr, gnr, bnc, bp = consts["bsr"], consts["gnr"], consts["bnc"], consts["bp"]

    with ExitStack() as ctx:
        pool_w = ctx.enter_context(tc.tile_pool(name="pool_w", bufs=1))
        pool_big = ctx.enter_context(tc.tile_pool(name="pool_big", bufs=1))
        pool_vw = ctx.enter_context(tc.tile_pool(name="pool_vw", bufs=2))
        pool_tmp = ctx.enter_context(tc.tile_pool(name="pool_tmp", bufs=2))

        # ---------------- weights / constants to SBUF ----------------
        wq, wp, wsr, wkv = [], [], [], []
        for ct in range(4):
            wq_t = pool_w.tile([128, C1], BF16, name=f"wq{ct}", tag=f"wq{ct}")
            nc.sync.dma_start(out=wq_t, in_=WqT[ct * 128:(ct + 1) * 128, :])
            wq.append(wq_t)
            wp_t = pool_w.tile([128, C1], BF16, name=f"wp{ct}", tag=f"wp{ct}")
            nc.sync.dma_start(out=wp_t, in_=WpT[ct * 128:(ct + 1) * 128, :])
            wp.append(wp_t)
        for kt in range(2):
            wsr_t = pool_w.tile([128, C2], BF16, name=f"wsr{kt}", tag=f"wsr{kt}")
            nc.sync.dma_start(out=wsr_t, in_=WsrT[kt * 128:(kt + 1) * 128, :])
            wsr.append(wsr_t)
            wkv_t = pool_w.tile([128, 2 * C1], BF16, name=f"wkv{kt}", tag=f"wkv{kt}")
            nc.sync.dma_start(out=wkv_t, in_=WkvT[kt * 128:(kt + 1) * 128, :])
            wkv.append(wkv_t)
        bsr_c, bn_c, gn_r = [], [], []
        for ot in range(2):
            b1 = pool_w.tile([128, 1], F32, name=f"bsr{ot}", tag=f"bsr{ot}")
            nc.sync.dma_start(out=b1, in_=bsr[ot * 128:(ot + 1) * 128].unsqueeze(1))
            bsr_c.append(b1)
            b2 = pool_w.tile([128, 1], F32, name=f"bn{ot}", tag=f"bn{ot}")
            nc.sync.dma_start(out=b2, in_=bnc[ot * 128:(ot + 1) * 128].unsqueeze(1))
            bn_c.append(b2)
            g0 = pool_w.tile([1, 128], F32, name=f"gnrf{ot}", tag=f"gnrf{ot}")
            nc.sync.dma_start(out=g0, in_=gnr[ot:ot + 1, :])
            g1 = pool_w.tile([1, 128], F32R, name=f"gnr{ot}", tag=f"gnr{ot}")
            nc.vector.tensor_copy(g1[:], g0[:])
            gn_r.append(g1)
        bp_sb = pool_w.tile([1, C1], BF16, name="bp_sb", tag="bp_sb")
        nc.sync.dma_start(out=bp_sb, in_=bp.ap())
        eye_sb = pool_w.tile([HP, HP], BF16, name="eye_sb", tag="eye_sb")
        nc.sync.dma_start(out=eye_sb, in_=consts["eye"].ap())
        # per-row dequant scales from the packed f32 rows (bitcast views),
        # transposed load: xs_all[p, r] = xs[r*56+p]
        fview = xyN.bitcast(F32)
        xflat = fview[N1:N1 + 25, :].rearrange("a b -> (a b)")
        xs_all = pool_w.tile([HP, HP], F32, name="xs_all", tag="xs_all")
        nc.sync.dma_start(out=xs_all,
                          in_=xflat[:N1].rearrange("(r p) -> p r", r=HP, p=HP))
        yflat = fview[4729:4754, :].rearrange("a b -> (a b)")
        ys_all = pool_w.tile([HP, HP], F32, name="ys_all", tag="ys_all")
        nc.sync.dma_start(out=ys_all,
                          in_=yflat[:N1].rearrange("(r p) -> p r", r=HP, p=HP))

        ones_f = pool_w.tile([128, 1], F32, name="ones_f", tag="ones_f")
        nc.vector.memset(ones_f, 1.0)
        ones_c = pool_w.tile([128, 1], F32R, name="ones_c", tag="ones_c")
        nc.vector.tensor_copy(ones_c[:], ones_f[:])
        ones_rf = pool_w.tile([1, 128], F32, name="ones_rf", tag="ones_rf")
        nc.vector.memset(ones_rf, 1.0)
        ones_r = pool_w.tile([1, 128], BF16, name="ones_r", tag="ones_r")
        nc.vector.tensor_copy(ones_r[:], ones_rf[:])
        ones_s = pool_w.tile([49, 64], BF16, name="ones_s", tag="ones_s")
        nc.vector.memset(ones_s, 1.0)
        eps_sb = pool_w.tile([1, 1], F32, name="eps_sb", tag="eps_sb")
        nc.vector.memset(eps_sb, EPS)

        # ---------------- persistent activations ----------------
        xT = [pool_big.tile([128, N1], BF16, name=f"xT{t}", tag=f"xT{t}")
              for t in range(4)]
        y2T = [pool_big.tile([128, N1], BF16, name=f"y2T{k}", tag=f"y2T{k}")
               for k in range(2)]
        kT = [pool_big.tile([128, N1], BF16, name=f"kT{t}", tag=f"kT{t}")
              for t in range(4)]
        qT = [pool_big.tile([128, N1], BF16, name=f"qT{t}", tag=f"qT{t}")
              for t in range(4)]

        def wdest(tile_, r):
            # window-major scatter view for spatial row r: [p, wj 8, j 7]
            # target col = (wi*8+wj)*49 + i*7 + j
            wi, i = r // WS, r % WS
            v = tile_.rearrange("p (a b i j) -> p a b i j", a=8, b=8, i=7, j=7)
            return v[:, wi, :, i]

        with tc.tile_pool(name="pool_yp", bufs=1) as pool_yp:
            ypT = [pool_yp.tile([128, N1], BF16, name=f"ypT{k}", tag=f"ypT{k}")
                   for k in range(2)]

            # ------------ stage 0: dequant + transpose to window-major ------------
            with tc.tile_pool(name="ps_t", bufs=1, space="PSUM") as ps_t:
                for r in range(HP):
                    rs_ = slice(r * HP, (r + 1) * HP)
                    xi = pool_tmp.tile([HP, C1], I8, name="xi", tag="xi", bufs=3)
                    nc.sync.dma_start(out=xi, in_=xN[rs_, :])
                    xb = pool_tmp.tile([HP, C1], BF16, name="xb", tag="xb", bufs=3)
                    nc.scalar.activation(out=xb[:], in_=xi[:], func=AF.Identity,
                                         scale=xs_all[:, r:r + 1])
                    yi = pool_tmp.tile([HP, C2], I8, name="yi", tag="yi", bufs=3)
                    nc.sync.dma_start(out=yi, in_=ypN[rs_, :])
                    yb = pool_tmp.tile([HP, C2], BF16, name="yb", tag="yb", bufs=3)
                    nc.scalar.activation(out=yb[:], in_=yi[:], func=AF.Identity,
                                         scale=ys_all[:, r:r + 1])
                    for cb in range(4):
                        pt = ps_t.tile([128, HP], F32, name="pt", tag="pt", bufs=6)
                        nc.tensor.matmul(pt[:], xb[:, cb * 128:(cb + 1) * 128],
                                         eye_sb[:], start=True, stop=True)
                        nc.vector.tensor_copy(
                            wdest(xT[cb], r),
                            pt.rearrange("p (b j) -> p b j", b=8, j=7))
                    for cb in range(2):
                        pt = ps_t.tile([128, HP], F32, name="pt2", tag="pt", bufs=6)
                        nc.tensor.matmul(pt[:], yb[:, cb * 128:(cb + 1) * 128],
                                         eye_sb[:], start=True, stop=True)
                        nc.vector.tensor_copy(
                            wdest(ypT[cb], r),
                            pt.rearrange("p (b j) -> p b j", b=8, j=7))

            ps_d_cm = tc.tile_pool(name="ps_d", bufs=2, space="PSUM")
            ps_d = ps_d_cm.__enter__()
            # ------------ stage 2: sr conv + LN + gelu ------------
            for ch in range(NCHUNKS):
                cs = slice(ch * NCH, (ch + 1) * NCH)
                zsb = []
                for ot in range(2):
                    pz = ps_d.tile([128, NCH], F32, name="pz", tag="pz")
                    for kt in range(2):
                        nc.tensor.matmul(pz[:], wsr[kt][:, ot * 128:(ot + 1) * 128],
                                         ypT[kt][:, cs],
                                         start=(kt == 0), stop=(kt == 1))
                    z_t = pool_tmp.tile([128, NCH], F32R, name="z_t",
                                        tag="zsb", bufs=4)
                    nc.scalar.activation(out=z_t[:], in_=pz[:], func=AF.Identity,
                                         bias=bsr_c[ot])
                    zsb.append(z_t)
                pst_s = ps_d.tile([1, NCH], F32, name="pst_s", tag="pst_s", bufs=1)
                pst_q = ps_d.tile([1, NCH], F32, name="pst_q", tag="pst_q", bufs=1)
                for ot in range(2):
                    nc.tensor.matmul(pst_s[:], ones_c[:], zsb[ot][:],
                                     start=(ot == 0), stop=(ot == 1))
                for ot in range(2):
                    zq = pool_tmp.tile([128, NCH], F32R, name="zq", tag="zq", bufs=2)
                    nc.scalar.activation(out=zq[:], in_=zsb[ot][:], func=AF.Square)
                    nc.tensor.matmul(pst_q[:], ones_c[:], zq[:],
                                     start=(ot == 0), stop=(ot == 1))
                m_sb = pool_tmp.tile([1, NCH], F32, name="m_sb", tag="m_sb", bufs=1)
                nc.vector.tensor_scalar_mul(m_sb[:], pst_s[:], 1.0 / C2)
                q_sb = pool_tmp.tile([1, NCH], F32, name="q_sb", tag="q_sb", bufs=1)
                nc.vector.tensor_scalar_mul(q_sb[:], pst_q[:], 1.0 / C2)
                var_sb = pool_tmp.tile([1, NCH], F32, name="var_sb",
                                       tag="var_sb", bufs=1)
                nc.gpsimd.tensor_tensor(var_sb[:], m_sb[:], m_sb[:],
                                        op=mybir.AluOpType.mult)
                nc.gpsimd.tensor_tensor(var_sb[:], q_sb[:], var_sb[:],
                                        op=mybir.AluOpType.subtract)
                sd_sb = pool_tmp.tile([1, NCH], F32, name="sd_sb",
                                      tag="sd_sb", bufs=1)
                nc.scalar.activation(out=sd_sb[:], in_=var_sb[:], func=AF.Sqrt,
                                     bias=eps_sb[:])
                r_sb = pool_tmp.tile([1, NCH], F32R, name="r_sb", tag="r_sb", bufs=1)
                with nc.allow_low_precision(reason="f32r rstd feeds f32r matmul"):
                    nc.vector.reciprocal(out=r_sb[:], in_=sd_sb[:])
                nb_sb = pool_tmp.tile([1, NCH], F32R, name="nb_sb",
                                      tag="nb_sb", bufs=1)
                nc.gpsimd.tensor_tensor(nb_sb[:], m_sb[:], r_sb[:],
                                        op=mybir.AluOpType.mult)
                nc.gpsimd.tensor_scalar_mul(nb_sb[:], nb_sb[:], -1.0)
                for ot in range(2):
                    pa = ps_d.tile([128, NCH], F32, name="pa", tag="pa")
                    nc.tensor.matmul(pa[:], gn_r[ot][:], r_sb[:],
                                     start=True, stop=True)
                    pb = ps_d.tile([128, NCH], F32, name="pb", tag="pb")
                    nc.tensor.matmul(pb[:], gn_r[ot][:], nb_sb[:],
                                     start=True, stop=True)
                    t1 = pool_tmp.tile([128, NCH], F32, name="t1", tag="t1", bufs=2)
                    nc.vector.tensor_mul(t1[:], zsb[ot][:], pa[:])
                    nc.vector.tensor_add(t1[:], t1[:], pb[:])
                    nc.scalar.activation(out=y2T[ot][:, cs], in_=t1[:],
                                         func=AF.Gelu, bias=bn_c[ot])

            # ------------ stage 3: k projection (channel-major) ------------
            for ch in range(NCHUNKS):
                cs = slice(ch * NCH, (ch + 1) * NCH)
                for ot in range(4):
                    pk = ps_d.tile([128, NCH], F32, name="pk", tag="pz")
                    for kt in range(2):
                        nc.tensor.matmul(pk[:],
                                         wkv[kt][:, ot * 128:(ot + 1) * 128],
                                         y2T[kt][:, cs],
                                         start=(kt == 0), stop=(kt == 1))
                    nc.any.tensor_copy(kT[ot][:, cs], pk[:])

            # ------------ stage 4: q projection (channel-major) ------------
            for ch in range(NCHUNKS):
                cs = slice(ch * NCH, (ch + 1) * NCH)
                for ot in range(4):
                    pq = ps_d.tile([128, NCH], F32, name="pq", tag="pz")
                    for ct in range(4):
                        nc.tensor.matmul(pq[:],
                                         wq[ct][:, ot * 128:(ot + 1) * 128],
                                         xT[ct][:, cs],
                                         start=(ct == 0), stop=(ct == 3))
                    nc.any.tensor_copy(qT[ot][:, cs], pq[:])
            ps_d_cm.__exit__(None, None, None)

        # ------------ stage 5-7: v (window-major), attention, proj ------------
        # qT/kT/y2T columns are window-major: window w = wi*8+wj occupies
        # cols w*49:(w+1)*49. attT stays spatial-major (scatter on write).

        def win_view(t):
            return t.rearrange("p (a i b j) -> p a b i j", a=8, i=7, b=8, j=7)

        with tc.tile_pool(name="pool_att", bufs=1) as pool_att, \
             tc.tile_pool(name="ps_a", bufs=2, space="PSUM") as ps_a:
            attT = [pool_att.tile([128, N1], BF16, name=f"attT{t}", tag=f"attT{t}")
                    for t in range(4)]
            for wi in range(8):
                vw = pool_vw.tile([49, 8 * C1], BF16, name="vw", tag="vw")
                for wj in range(8):
                    wsl = slice((wi * 8 + wj) * 49, (wi * 8 + wj + 1) * 49)
                    pv = ps_a.tile([49, C1], F32, name="pv", tag="pv")
                    for kt in range(2):
                        nc.tensor.matmul(pv[:], y2T[kt][:, wsl],
                                         wkv[kt][:, C1:2 * C1],
                                         start=(kt == 0), stop=(kt == 1))
                    nc.scalar.copy(out=vw[:, wj * C1:(wj + 1) * C1], in_=pv[:])
                for h in range(8):
                    t, pb_ = h // 2, (h % 2) * 64
                    psl = slice(pb_, pb_ + 64)
                    S = ps_a.tile([49, 392], F32, name="S", tag="S")
                    for wj in range(8):
                        wsl = slice((wi * 8 + wj) * 49, (wi * 8 + wj + 1) * 49)
                        nc.tensor.matmul(S[:, wj * 49:(wj + 1) * 49],
                                         kT[t][psl, wsl],
                                         qT[t][psl, wsl],
                                         start=True, stop=True)
                    E = pool_tmp.tile([49, 392], BF16, name="E", tag="E", bufs=3)
                    nc.scalar.activation(out=E[:], in_=S[:], func=AF.Exp,
                                         scale=0.125)
                    SUMB = ps_a.tile([64, 392], F32, name="SUMB",
                                     tag="SUMB", bufs=1)
                    nc.tensor.matmul(SUMB[:], ones_s[:], E[:],
                                     start=True, stop=True)
                    RB = pool_tmp.tile([64, 392], F32, name="RB", tag="RB", bufs=3)
                    nc.vector.reciprocal(out=RB[:], in_=SUMB[:])
                    AV = ps_a.tile([64, 392], F32, name="AV", tag="AV")
                    for wj in range(8):
                        nc.tensor.matmul(
                            AV[:, wj * 49:(wj + 1) * 49],
                            vw[:, wj * C1 + h * 64:wj * C1 + (h + 1) * 64],
                            E[:, wj * 49:(wj + 1) * 49],
                            start=True, stop=True)
                    avv = AV.rearrange("p (b i j) -> p b i j", b=8, i=7, j=7)
                    rbv = RB.rearrange("p (b i j) -> p b i j", b=8, i=7, j=7)
                    nc.vector.tensor_mul(win_view(attT[t])[psl, wi],
                                         avv[:], rbv[:])

            # ------------ stage 7: output projection + int8 quantization ------------
            for nt in range(NT):
                nsz = min(128, N1 - nt * 128)
                ns = slice(nt * 128, nt * 128 + nsz)
                po = ps_a.tile([128, C1], F32, name="po", tag="pv")
                for ct in range(4):
                    nc.tensor.matmul(po[:nsz, :], attT[ct][:, ns], wp[ct][:],
                                     start=(ct == 0), stop=False)
                nc.tensor.matmul(po[:nsz, :], ones_r[:, :nsz], bp_sb[:],
                                 start=False, stop=True)
                # per-row absmax -> int8 scale; conversion rounds-to-nearest
                mx = pool_tmp.tile([128, 1], F32, name="mx", tag="mx", bufs=2)
                nc.vector.tensor_reduce(mx[:nsz, :], po[:nsz, :],
                                        axis=mybir.AxisListType.X,
                                        op=mybir.AluOpType.max,
                                        apply_absolute_value=True)
                nc.vector.tensor_scalar_max(mx[:nsz, :], mx[:nsz, :], 1e-30)
                rs = pool_tmp.tile([128, 1], F32, name="rs", tag="rs", bufs=2)
                nc.vector.reciprocal(out=rs[:nsz, :], in_=mx[:nsz, :])
                nc.vector.tensor_scalar_mul(rs[:nsz, :], rs[:nsz, :], 127.0)
                o_i8 = pool_tmp.tile([128, C1], I8, name="o_i8",
                                     tag="o_i8", bufs=2)
                nc.scalar.activation(out=o_i8[:nsz, :], in_=po[:nsz, :],
                                     func=AF.Identity, scale=rs[:nsz, :])
                nc.sync.dma_start(out=out[ns, :], in_=o_i8[:nsz, :])
                nc.sync.dma_start(out=out.bitcast(F32)[N1 + nt:N1 + nt + 1, :nsz],
                                  in_=mx[:nsz, :])


def _get_nc(w):
    rebuild = True
    if "nc" in _cache:
        old = _cache["w"]
        rebuild = not all(np.array_equal(old[k], w[k]) for k in old)
    if rebuild:
        _cache["nc"] = _build_nc(w)
        _cache["w"] = w
    return _cache["nc"]


def _quant_rows(a, pow2=1.0):
    """Per-row symmetric int8: returns (int8 array, dequant scale per row)."""
    s = np.abs(a).max(-1, keepdims=True)
    np.maximum(s, 1e-30, out=s)
    t = a * (127.0 / s)
    t += 128.5
    u = t.astype(np.uint8)
    np.bitwise_xor(u, 128, out=u)
    return u.view(np.int8), (s * (1.0 / (127.0 * pow2))).astype(np.float32)


def _executor():
    if "pool" not in _cache:
        from concurrent.futures import ThreadPoolExecutor
        _cache["pool"] = ThreadPoolExecutor(max_workers=4)
    return _cache["pool"]


def kernel(**inputs):
    import ml_dtypes
    bf16 = ml_dtypes.bfloat16
    f32 = np.float32

    x = np.asarray(inputs["x"], dtype=f32)
    y = np.asarray(inputs["y"], dtype=f32)
    Wq = np.asarray(inputs["Wq"], dtype=f32)
    Wkv = np.asarray(inputs["Wkv"], dtype=f32)
    Wproj = np.asarray(inputs["Wproj"], dtype=f32)
    bproj = np.asarray(inputs["bproj"], dtype=f32)
    bsr_np = np.asarray(inputs["bsr"], dtype=f32)
    Wsr = np.asarray(inputs["Wsr"], dtype=f32)
    gn = np.asarray(inputs["gn"], dtype=f32)
    bn = np.asarray(inputs["bn"], dtype=f32)

    w = {
        "WqT": np.ascontiguousarray(Wq.T).astype(bf16),
        "WsrT": np.ascontiguousarray(Wsr.T).astype(bf16),
        "WkvT": np.ascontiguousarray(Wkv.T).astype(bf16),
        "WpT": np.ascontiguousarray(Wproj.T).astype(bf16),
        "bsr": bsr_np,
        "gnr": np.ascontiguousarray(gn.reshape(2, 128)).astype(f32),
        "bnc": bn,
        "bp": np.ascontiguousarray(bproj.reshape(1, C1)).astype(bf16),
        "eye": np.eye(HP, dtype=bf16),
    }

    # x: per-row int8 (natural layout; device transposes + window-majors)
    # y: 2x2 sum-pool on host, per-row int8; /4 (pool mean) folded into scales
    # The f32 dequant scales are packed into extra int8 rows of each buffer.
    xybuf = np.empty((B, N1 + 25 + 1593, C1), np.int8)
    xbuf = xybuf[:, :N1 + 25, :]
    ybuf = xybuf[:, N1 + 25:, :].reshape(B, 1593 * C1)[:, :(N1 + 49) * C2] \
                .reshape(B, N1 + 49, C2)

    def pack(buf, bs, i8, sc):
        np.copyto(buf[bs, :N1, :], i8)
        for k, b in enumerate(range(bs.start, bs.stop)):
            buf[b, N1:].reshape(-1).view(np.float32)[:N1] = sc[k, :, 0]

    def quant_x(bs):
        i8, sc = _quant_rows(x.reshape(B, N1, C1)[bs])
        pack(xbuf, bs, i8, sc)

    def quant_y(bs):
        yb = y.reshape(B, H2, HP, 2, C2)[bs]
        s1 = yb.sum(3)
        n = s1.shape[0]
        ysum = s1.reshape(n, HP, 2, HP, C2).sum(2).reshape(n, N1, C2)
        i8, sc = _quant_rows(ysum, pow2=4.0)
        pack(ybuf, bs, i8, sc)

    ex = _executor()
    h = slice(0, B // 2)
    t = slice(B // 2, B)
    futs = [ex.submit(quant_x, h), ex.submit(quant_x, t),
            ex.submit(quant_y, h), ex.submit(quant_y, t)]
    for f in futs:
        f.result()

    nc = _get_nc(w)
    in_maps = [{"xyN": xybuf[b]} for b in range(B)]
    from concourse.bass_utils import run_bass_kernel_spmd
    res = run_bass_kernel_spmd(nc, in_maps, core_ids=list(range(B)),
                               **_cache.get("run_opts", {}))
    _cache["last_res"] = res

    # dequantize: scale for out row n is packed f32 element n of the tail rows
    outs = np.stack([r["out"] for r in res.results], axis=0)  # (B, N1+25, C1) i8
    i8 = outs[:, :N1, :]
    srow = np.ascontiguousarray(outs[:, N1:, :]).reshape(B, -1) \
             .view(np.float32)[:, :N1] * (1.0 / 127.0)
    return i8 * srow[:, :, None]


# revision 18
# speedup vs baseline: 3.8676x; 1.0143x over previous
"""Trainium2 Bass kernel for windowed cross-attention (nn_CrossAttention_37056977830404).

Sharding: data-parallel over batch B=8 across the 8 NeuronCores (one batch
element per core). The call is transfer-bound over the axon tunnel
(~40 MB/s in, ~28 MB/s out), so the design minimizes bytes moved:
  - weights are baked into the NEFF as Const tensors (loaded once),
  - y is 2x2 sum-pooled on the host (4x reduction),
  - x and pooled-y are sent as int8 with per-row scales (2x vs bf16),
  - the output is int8 with per-row scales (4x smaller than f32, and the
    donated zero output buffers the PJRT path uploads shrink the same way).

Per-core pipeline (all shapes hardcoded):
  stage 0: xN [3136,512] i8, ypN [3136,256] i8 (natural layout) are
  dequantized to bf16 (per-row scales) and transposed on the tensor engine
  (identity matmuls, 56-row chunks) into window-major channel-major SBUF
  tiles: col n' = (wi*8+wj)*49 + i*7 + j.
  z = yp @ Wsr.T + bsr  (bf16 matmul, fp32 psum)     [sr conv; /4 in scales]
  LN over channels (cross-partition ones-matmul sums) + gelu -> y2T bf16
  kT = (y2 @ Wkv_k.T).T     [channel-major, bf16]
  v_w = y2 @ Wkv_v.T        [window-major via windowed stationary APs, bf16]
  qT = (x @ Wq.T).T         [channel-major, bf16]
  per (head, window-row): S^T = k_w^T q_w ; E = exp(S^T/8) ; sums via
  ones-matmul broadcast ; AV = v_w^T E ; attT = AV * recip(sum)  [bf16]
  out = attT.T @ Wproj.T + bproj  (bf16 matmuls), then per-row int8
  quantization: scl = absmax(row), out_i8 = round(out * 127/scl).
"""
import os
import sys

sys.path.insert(0, '/opt/trn_rl_repo')
os.environ.setdefault("JAX_COMPILATION_CACHE_DIR", "/tmp/jax_ccache")
os.environ.setdefault("JAX_PERSISTENT_CACHE_MIN_COMPILE_TIME_SECS", "0")
os.environ.setdefault("JAX_PERSISTENT_CACHE_MIN_ENTRY_SIZE_BYTES", "0")
import numpy as np

try:  # env vars above are too late if jax was imported first; force via config
    import jax as _jax
    _jax.config.update("jax_compilation_cache_dir", "/tmp/jax_ccache")
    _jax.config.update("jax_persistent_cache_min_compile_time_secs", 0)
    _jax.config.update("jax_persistent_cache_min_entry_size_bytes", 0)
except Exception:
    pass

B = 8
C1 = 512
N1 = 3136
NH = 8
HD = 64
WS = 7
C2 = 256
H2 = W2 = 112
HP = WP = 56
NCH = 392      # dense matmul n-chunk (free dim) = one window-row
NCHUNKS = 8    # 3136 / 392
NT = 25        # output row tiles (24x128 + 64)
EPS = 1e-5

_cache = {}


def _build_nc(w):
    import concourse.bacc as bacc
    import concourse.tile as tile
    from concourse import mybir

    F32 = mybir.dt.float32
    BF16 = mybir.dt.bfloat16
    I8 = mybir.dt.int8

    nc = bacc.Bacc()

    # ---------------- DRAM I/O ----------------
    # One packed int8 input: x rows [0,3161) (3136 data + 25 f32-scale rows),
    # then the pooled-y region (3136 x 256 data + 49 x 256 scale bytes) packed
    # two 256B rows per 512B row, 1593 rows, 256B tail pad. One tensor ->
    # one transfer latency + one concat.
    xyN = nc.dram_tensor("xyN", [N1 + 25 + 1593, C1], I8, kind="ExternalInput")
    out = nc.dram_tensor("out", [N1 + 25, C1], I8, kind="ExternalOutput")
    # weights baked into the NEFF (DMA'd to HBM once at model load)
    consts = {
        "WqT": nc.inline_tensor(w["WqT"], name="cWqT"),
        "WsrT": nc.inline_tensor(w["WsrT"], name="cWsrT"),
        "WkvT": nc.inline_tensor(w["WkvT"], name="cWkvT"),
        "WpT": nc.inline_tensor(w["WpT"], name="cWpT"),
        "bsr": nc.inline_tensor(w["bsr"], name="cbsr"),
        "gnr": nc.inline_tensor(w["gnr"], name="cgnr"),
        "bnc": nc.inline_tensor(w["bnc"], name="cbnc"),
        "bp": nc.inline_tensor(w["bp"], name="cbp"),
        "eye": nc.inline_tensor(w["eye"], name="ceye"),
    }

    with tile.TileContext(nc) as tc:
        _emit(nc, tc, mybir, F32, BF16, I8, xyN, consts, out)
    nc.finalize()
    return nc


def _emit(nc, tc, mybir, F32, BF16, I8, xyN, consts, out):
    xN = xyN  # x region: rows [0, N1+25)
    ypN = xyN.ap().rearrange("a (h c) -> (a h) c", h=2, c=C2)[2 * (N1 + 25):]
    from contextlib import ExitStack

    F32R = mybir.dt.float32r
    AF = mybir.ActivationFunctionType
    WqT, WsrT, WkvT, WpT = (consts["WqT"], consts["WsrT"], consts["WkvT"],
                            consts["WpT"])
    bsr, gnr, bnc, bp = consts["bsr"], consts["gnr"], consts["bnc"], consts["bp"]

    with ExitStack() as ctx:
        pool_w = ctx.enter_context(tc.tile_pool(name="pool_w", bufs=1))
        pool_big = ctx.enter_context(tc.tile_pool(name="pool_big", bufs=1))
        pool_vw = ctx.enter_context(tc.tile_pool(name="pool_vw", bufs=2))
        pool_tmp = ctx.enter_context(tc.tile_pool(name="pool_tmp", bufs=2))

        # ---------------- weights / constants to SBUF ----------------
        wq, wp, wsr, wkv = [], [], [], []
        for ct in range(4):
            wq_t = pool_w.tile([128, C1], BF16, name=f"wq{ct}", tag=f"wq{ct}")
            nc.sync.dma_start(out=wq_t, in_=WqT[ct * 128:(ct + 1) * 128, :])
            wq.append(wq_t)
            wp_t = pool_w.tile([128, C1], BF16, name=f"wp{ct}", tag=f"wp{ct}")
            nc.sync.dma_start(out=wp_t, in_=WpT[ct * 128:(ct + 1) * 128, :])
            wp.append(wp_t)
        for kt in range(2):
            wsr_t = pool_w.tile([128, C2], BF16, name=f"wsr{kt}", tag=f"wsr{kt}")
            nc.sync.dma_start(out=wsr_t, in_=WsrT[kt * 128:(kt + 1) * 128, :])
            wsr.append(wsr_t)
            wkv_t = pool_w.tile([128, 2 * C1], BF16, name=f"wkv{kt}", tag=f"wkv{kt}")
            nc.sync.dma_start(out=wkv_t, in_=WkvT[kt * 128:(kt + 1) * 128, :])
            wkv.append(wkv_t)
        bsr_c, bn_c, gn_r = [], [], []
        for ot in range(2):
            b1 = pool_w.tile([128, 1], F32, name=f"bsr{ot}", tag=f"bsr{ot}")
            nc.sync.dma_start(out=b1, in_=bsr[ot * 128:(ot + 1) * 128].unsqueeze(1))
            bsr_c.append(b1)
            b2 = pool_w.tile([128, 1], F32, name=f"bn{ot}", tag=f"bn{ot}")
            nc.sync.dma_start(out=b2, in_=bnc[ot * 128:(ot + 1) * 128].unsqueeze(1))
            bn_c.append(b2)
            g0 = pool_w.tile([1, 128], F32, name=f"gnrf{ot}", tag=f"gnrf{ot}")
            nc.sync.dma_start(out=g0, in_=gnr[ot:ot + 1, :])
            g1 = pool_w.tile([1, 128], F32R, name=f"gnr{ot}", tag=f"gnr{ot}")
            nc.vector.tensor_copy(g1[:], g0[:])
            gn_r.append(g1)
        bp_sb = pool_w.tile([1, C1], BF16, name="bp_sb", tag="bp_sb")
        nc.sync.dma_start(out=bp_sb, in_=bp.ap())
        eye_sb = pool_w.tile([HP, HP], BF16, name="eye_sb", tag="eye_sb")
        nc.sync.dma_start(out=eye_sb, in_=consts["eye"].ap())
        # per-row dequant scales from the packed f32 rows (bitcast views),
        # transposed load: xs_all[p, r] = xs[r*56+p]
        fview = xyN.bitcast(F32)
        xflat = fview[N1:N1 + 25, :].rearrange("a b -> (a b)")
        xs_all = pool_w.tile([HP, HP], F32, name="xs_all", tag="xs_all")
        nc.sync.dma_start(out=xs_all,
                          in_=xflat[:N1].rearrange("(r p) -> p r", r=HP, p=HP))
        yflat = fview[4729:4754, :].rearrange("a b -> (a b)")
        ys_all = pool_w.tile([HP, HP], F32, name="ys_all", tag="ys_all")
        nc.sync.dma_start(out=ys_all,
                          in_=yflat[:N1].rearrange("(r p) -> p r", r=HP, p=HP))

        ones_f = pool_w.tile([128, 1], F32, name="ones_f", tag="ones_f")
        nc.vector.memset(ones_f, 1.0)
        ones_c = pool_w.tile([128, 1], F32R, name="ones_c", tag="ones_c")
        nc.vector.tensor_copy(ones_c[:], ones_f[:])
        ones_rf = pool_w.tile([1, 128], F32, name="ones_rf", tag="ones_rf")
        nc.vector.memset(ones_rf, 1.0)
        ones_r = pool_w.tile([1, 128], BF16, name="ones_r", tag="ones_r")
        nc.vector.tensor_copy(ones_r[:], ones_rf[:])
        ones_s = pool_w.tile([49, 64], BF16, name="ones_s", tag="ones_s")
        nc.vector.memset(ones_s, 1.0)
        eps_sb = pool_w.tile([1, 1], F32, name="eps_sb", tag="eps_sb")
        nc.vector.memset(eps_sb, EPS)

        # ---------------- persistent activations ----------------
        xT = [pool_big.tile([128, N1], BF16, name=f"xT{t}", tag=f"xT{t}")
              for t in range(4)]
        y2T = [pool_big.tile([128, N1], BF16, name=f"y2T{k}", tag=f"y2T{k}")
               for k in range(2)]
        kT = [pool_big.tile([128, N1], BF16, name=f"kT{t}", tag=f"kT{t}")
              for t in range(4)]
        qT = [pool_big.tile([128, N1], BF16, name=f"qT{t}", tag=f"qT{t}")
              for t in range(4)]

        def wdest(tile_, r):
            # window-major scatter view for spatial row r: [p, wj 8, j 7]
            # target col = (wi*8+wj)*49 + i*7 + j
            wi, i = r // WS, r % WS
            v = tile_.rearrange("p (a b i j) -> p a b i j", a=8, b=8, i=7, j=7)
            return v[:, wi, :, i]

        with tc.tile_pool(name="pool_yp", bufs=1) as pool_yp:
            ypT = [pool_yp.tile([128, N1], BF16, name=f"ypT{k}", tag=f"ypT{k}")
                   for k in range(2)]

            # ------------ stage 0: dequant + transpose to window-major ------------
            with tc.tile_pool(name="ps_t", bufs=1, space="PSUM") as ps_t:
                for r in range(HP):
                    rs_ = slice(r * HP, (r + 1) * HP)
                    xi = pool_tmp.tile([HP, C1], I8, name="xi", tag="xi", bufs=3)
                    nc.sync.dma_start(out=xi, in_=xN[rs_, :])
                    xb = pool_tmp.tile([HP, C1], BF16, name="xb", tag="xb", bufs=3)
                    nc.scalar.activation(out=xb[:], in_=xi[:], func=AF.Identity,
                                         scale=xs_all[:, r:r + 1])
                    yi = pool_tmp.tile([HP, C2], I8, name="yi", tag="yi", bufs=3)
                    nc.sync.dma_start(out=yi, in_=ypN[rs_, :])
                    yb = pool_tmp.tile([HP, C2], BF16, name="yb", tag="yb", bufs=3)
                    nc.scalar.activation(out=yb[:], in_=yi[:], func=AF.Identity,
                                         scale=ys_all[:, r:r + 1])
                    for cb in range(4):
                        pt = ps_t.tile([128, HP], F32, name="pt", tag="pt", bufs=6)
                        nc.tensor.matmul(pt[:], xb[:, cb * 128:(cb + 1) * 128],
                                         eye_sb[:], start=True, stop=True)
                        nc.vector.tensor_copy(
                            wdest(xT[cb], r),
                            pt.rearrange("p (b j) -> p b j", b=8, j=7))
                    for cb in range(2):
                        pt = ps_t.tile([128, HP], F32, name="pt2", tag="pt", bufs=6)
                        nc.tensor.matmul(pt[:], yb[:, cb * 128:(cb + 1) * 128],
                                         eye_sb[:], start=True, stop=True)
                        nc.vector.tensor_copy(
                            wdest(ypT[cb], r),
                            pt.rearrange("p (b j) -> p b j", b=8, j=7))

            ps_d_cm = tc.tile_pool(name="ps_d", bufs=2, space="PSUM")
            ps_d = ps_d_cm.__enter__()
            # ------------ stage 2: sr conv + LN + gelu ------------
            for ch in range(NCHUNKS):
                cs = slice(ch * NCH, (ch + 1) * NCH)
                zsb = []
                for ot in range(2):
                    pz = ps_d.tile([128, NCH], F32, name="pz", tag="pz")
                    for kt in range(2):
                        nc.tensor.matmul(pz[:], wsr[kt][:, ot * 128:(ot + 1) * 128],
                                         ypT[kt][:, cs],
                                         start=(kt == 0), stop=(kt == 1))
                    z_t = pool_tmp.tile([128, NCH], F32R, name="z_t",
                                        tag="zsb", bufs=4)
                    nc.scalar.activation(out=z_t[:], in_=pz[:], func=AF.Identity,
                                         bias=bsr_c[ot])
                    zsb.append(z_t)
                pst_s = ps_d.tile([1, NCH], F32, name="pst_s", tag="pst_s", bufs=1)
                pst_q = ps_d.tile([1, NCH], F32, name="pst_q", tag="pst_q", bufs=1)
                for ot in range(2):
                    nc.tensor.matmul(pst_s[:], ones_c[:], zsb[ot][:],
                                     start=(ot == 0), stop=(ot == 1))
                for ot in range(2):
                    zq = pool_tmp.tile([128, NCH], F32R, name="zq", tag="zq", bufs=2)
                    nc.scalar.activation(out=zq[:], in_=zsb[ot][:], func=AF.Square)
                    nc.tensor.matmul(pst_q[:], ones_c[:], zq[:],
                                     start=(ot == 0), stop=(ot == 1))
                m_sb = pool_tmp.tile([1, NCH], F32, name="m_sb", tag="m_sb", bufs=1)
                nc.vector.tensor_scalar_mul(m_sb[:], pst_s[:], 1.0 / C2)
                q_sb = pool_tmp.tile([1, NCH], F32, name="q_sb", tag="q_sb", bufs=1)
                nc.vector.tensor_scalar_mul(q_sb[:], pst_q[:], 1.0 / C2)
                var_sb = pool_tmp.tile([1, NCH], F32, name="var_sb",
                                       tag="var_sb", bufs=1)
                nc.gpsimd.tensor_tensor(var_sb[:], m_sb[:], m_sb[:],
                                        op=mybir.AluOpType.mult)
                nc.gpsimd.tensor_tensor(var_sb[:], q_sb[:], var_sb[:],
                                        op=mybir.AluOpType.subtract)
                sd_sb = pool_tmp.tile([1, NCH], F32, name="sd_sb",
                                      tag="sd_sb", bufs=1)
                nc.scalar.activation(out=sd_sb[:], in_=var_sb[:], func=AF.Sqrt,
                                     bias=eps_sb[:])
                r_sb = pool_tmp.tile([1, NCH], F32R, name="r_sb", tag="r_sb", bufs=1)
                with nc.allow_low_precision(reason="f32r rstd feeds f32r matmul"):
                    nc.vector.reciprocal(out=r_sb[:], in_=sd_sb[:])
                nb_sb = pool_tmp.tile([1, NCH], F32R, name="nb_sb",
                                      tag="nb_sb", bufs=1)
                nc.gpsimd.tensor_tensor(nb_sb[:], m_sb[:], r_sb[:],
                                        op=mybir.AluOpType.mult)
                nc.gpsimd.tensor_scalar_mul(nb_sb[:], nb_sb[:], -1.0)
                for ot in range(2):
                    pa = ps_d.tile([128, NCH], F32, name="pa", tag="pa")
                    nc.tensor.matmul(pa[:], gn_r[ot][:], r_sb[:],
                                     start=True, stop=True)
                    pb = ps_d.tile([128, NCH], F32, name="pb", tag="pb")
                    nc.tensor.matmul(pb[:], gn_r[ot][:], nb_sb[:],
                                     start=True, stop=True)
                    t1 = pool_tmp.tile([128, NCH], F32, name="t1", tag="t1", bufs=2)
                    nc.vector.tensor_mul(t1[:], zsb[ot][:], pa[:])
                    nc.vector.tensor_add(t1[:], t1[:], pb[:])
                    nc.scalar.activation(out=y2T[ot][:, cs], in_=t1[:],
                                         func=AF.Gelu, bias=bn_c[ot])

            # ------------ stage 3: k projection (channel-major) ------------
            for ch in range(NCHUNKS):
                cs = slice(ch * NCH, (ch + 1) * NCH)
                for ot in range(4):
                    pk = ps_d.tile([128, NCH], F32, name="pk", tag="pz")
                    for kt in range(2):
                        nc.tensor.matmul(pk[:],
                                         wkv[kt][:, ot * 128:(ot + 1) * 128],
                                         y2T[kt][:, cs],
                                         start=(kt == 0), stop=(kt == 1))
                    nc.any.tensor_copy(kT[ot][:, cs], pk[:])

            # ------------ stage 4: q projection (channel-major) ------------
            for ch in range(NCHUNKS):
                cs = slice(ch * NCH, (ch + 1) * NCH)
                for ot in range(4):
                    pq = ps_d.tile([128, NCH], F32, name="pq", tag="pz")
                    for ct in range(4):
                        nc.tensor.matmul(pq[:],
                                         wq[ct][:, ot * 128:(ot + 1) * 128],
                                         xT[ct][:, cs],
                                         start=(ct == 0), stop=(ct == 3))
                    nc.any.tensor_copy(qT[ot][:, cs], pq[:])
            ps_d_cm.__exit__(None, None, None)

        # ------------ stage 5-7: v (window-major), attention, proj ------------
        # qT/kT/y2T columns are window-major: window w = wi*8+wj occupies
        # cols w*49:(w+1)*49. attT stays spatial-major (scatter on write).

        def win_view(t):
            return t.rearrange("p (a i b j) -> p a b i j", a=8, i=7, b=8, j=7)

        with tc.tile_pool(name="pool_att", bufs=1) as pool_att, \
             tc.tile_pool(name="ps_a", bufs=2, space="PSUM") as ps_a:
            attT = [pool_att.tile([128, N1], BF16, name=f"attT{t}", tag=f"attT{t}")
                    for t in range(4)]
            for wi in range(8):
                vw = pool_vw.tile([49, 8 * C1], BF16, name="vw", tag="vw")
                for wj in range(8):
                    wsl = slice((wi * 8 + wj) * 49, (wi * 8 + wj + 1) * 49)
                    pv = ps_a.tile([49, C1], F32, name="pv", tag="pv")
                    for kt in range(2):
                        nc.tensor.matmul(pv[:], y2T[kt][:, wsl],
                                         wkv[kt][:, C1:2 * C1],
                                         start=(kt == 0), stop=(kt == 1))
                    nc.scalar.copy(out=vw[:, wj * C1:(wj + 1) * C1], in_=pv[:])
                for h in range(8):
                    t, pb_ = h // 2, (h % 2) * 64
                    psl = slice(pb_, pb_ + 64)
                    S = ps_a.tile([49, 392], F32, name="S", tag="S")
                    for wj in range(8):
                        wsl = slice((wi * 8 + wj) * 49, (wi * 8 + wj + 1) * 49)
                        nc.tensor.matmul(S[:, wj * 49:(wj + 1) * 49],
                                         kT[t][psl, wsl],
                                         qT[t][psl, wsl],
                                         start=True, stop=True)
                    E = pool_tmp.tile([49, 392], BF16, name="E", tag="E", bufs=3)
                    nc.scalar.activation(out=E[:], in_=S[:], func=AF.Exp,
                                         scale=0.125)
                    SUMB = ps_a.tile([64, 392], F32, name="SUMB",
                                     tag="SUMB", bufs=1)
                    nc.tensor.matmul(SUMB[:], ones_s[:], E[:],
                                     start=True, stop=True)
                    RB = pool_tmp.tile([64, 392], F32, name="RB", tag="RB", bufs=3)
                    nc.vector.reciprocal(out=RB[:], in_=SUMB[:])
                    AV = ps_a.tile([64, 392], F32, name="AV", tag="AV")
                    for wj in range(8):
                        nc.tensor.matmul(
                            AV[:, wj * 49:(wj + 1) * 49],
                            vw[:, wj * C1 + h * 64:wj * C1 + (h + 1) * 64],
                            E[:, wj * 49:(wj + 1) * 49],
                            start=True, stop=True)
                    avv = AV.rearrange("p (b i j) -> p b i j", b=8, i=7, j=7)
                    rbv = RB.rearrange("p (b i j) -> p b i j", b=8, i=7, j=7)
                    nc.vector.tensor_mul(win_view(attT[t])[psl, wi],
                                         avv[:], rbv[:])

            # ------------ stage 7: output projection + int8 quantization ------------
            for nt in range(NT):
                nsz = min(128, N1 - nt * 128)
                ns = slice(nt * 128, nt * 128 + nsz)
                po = ps_a.tile([128, C1], F32, name="po", tag="pv")
                for ct in range(4):
                    nc.tensor.matmul(po[:nsz, :], attT[ct][:, ns], wp[ct][:],
                                     start=(ct == 0), stop=False)
                nc.tensor.matmul(po[:nsz, :], ones_r[:, :nsz], bp_sb[:],
                                 start=False, stop=True)
                # per-row absmax -> int8 scale; conversion rounds-to-nearest
                mx = pool_tmp.tile([128, 1], F32, name="mx", tag="mx", bufs=2)
                nc.vector.tensor_reduce(mx[:nsz, :], po[:nsz, :],
                                        axis=mybir.AxisListType.X,
                                        op=mybir.AluOpType.max,
                                        apply_absolute_value=True)
                nc.vector.tensor_scalar_max(mx[:nsz, :], mx[:nsz, :], 1e-30)
                rs = pool_tmp.tile([128, 1], F32, name="rs", tag="rs", bufs=2)
                nc.vector.reciprocal(out=rs[:nsz, :], in_=mx[:nsz, :])
                nc.vector.tensor_scalar_mul(rs[:nsz, :], rs[:nsz, :], 127.0)
                o_i8 = pool_tmp.tile([128, C1], I8, name="o_i8",
                                     tag="o_i8", bufs=2)
                nc.scalar.activation(out=o_i8[:nsz, :], in_=po[:nsz, :],
                                     func=AF.Identity, scale=rs[:nsz, :])
                nc.sync.dma_start(out=out[ns, :], in_=o_i8[:nsz, :])
                nc.sync.dma_start(out=out.bitcast(F32)[N1 + nt:N1 + nt + 1, :nsz],
                                  in_=mx[:nsz, :])


def _get_nc(w):
    rebuild = True
    if "nc" in _cache:
        old = _cache["w"]
        rebuild = not all(np.array_equal(old[k], w[k]) for k in old)
    if rebuild:
        _cache["nc"] = _build_nc(w)
        _cache["w"] = w
    return _cache["nc"]


def _quant_rows(a, pow2=1.0):
    """Per-row symmetric int8: returns (int8 array, dequant scale per row)."""
    s = np.abs(a).max(-1, keepdims=True)
    np.maximum(s, 1e-30, out=s)
    t = a * (127.0 / s)
    t += 128.5
    u = t.astype(np.uint8)
    np.bitwise_xor(u, 128, out=u)
    return u.view(np.int8), (s * (1.0 / (127.0 * pow2))).astype(np.float32)


def _executor():
    if "pool" not in _cache:
        from concurrent.futures import ThreadPoolExecutor
        _cache["pool"] = ThreadPoolExecutor(max_workers=8)
    return _cache["pool"]


def kernel(**inputs):
    import ml_dtypes
    bf16 = ml_dtypes.bfloat16
    f32 = np.float32

    x = np.asarray(inputs["x"], dtype=f32)
    y = np.asarray(inputs["y"], dtype=f32)
    Wq = np.asarray(inputs["Wq"], dtype=f32)
    Wkv = np.asarray(inputs["Wkv"], dtype=f32)
    Wproj = np.asarray(inputs["Wproj"], dtype=f32)
    bproj = np.asarray(inputs["bproj"], dtype=f32)
    bsr_np = np.asarray(inputs["bsr"], dtype=f32)
    Wsr = np.asarray(inputs["Wsr"], dtype=f32)
    gn = np.asarray(inputs["gn"], dtype=f32)
    bn = np.asarray(inputs["bn"], dtype=f32)

    w = {
        "WqT": np.ascontiguousarray(Wq.T).astype(bf16),
        "WsrT": np.ascontiguousarray(Wsr.T).astype(bf16),
        "WkvT": np.ascontiguousarray(Wkv.T).astype(bf16),
        "WpT": np.ascontiguousarray(Wproj.T).astype(bf16),
        "bsr": bsr_np,
        "gnr": np.ascontiguousarray(gn.reshape(2, 128)).astype(f32),
        "bnc": bn,
        "bp": np.ascontiguousarray(bproj.reshape(1, C1)).astype(bf16),
        "eye": np.eye(HP, dtype=bf16),
    }

    # x: per-row int8 (natural layout; device transposes + window-majors)
    # y: 2x2 sum-pool on host, per-row int8; /4 (pool mean) folded into scales
    # The f32 dequant scales are packed into extra int8 rows of each buffer.
    xybuf = np.empty((B, N1 + 25 + 1593, C1), np.int8)
    xbuf = xybuf[:, :N1 + 25, :]
    ybuf = xybuf[:, N1 + 25:, :].reshape(B, 1593 * C1)[:, :(N1 + 49) * C2] \
                .reshape(B, N1 + 49, C2)

    def pack(buf, bs, i8, sc):
        np.copyto(buf[bs, :N1, :], i8)
        for k, b in enumerate(range(bs.start, bs.stop)):
            buf[b, N1:].reshape(-1).view(np.float32)[:N1] = sc[k, :, 0]

    def quant_x(bs):
        i8, sc = _quant_rows(x.reshape(B, N1, C1)[bs])
        pack(xbuf, bs, i8, sc)

    def quant_y(bs):
        yb = y.reshape(B, H2, HP, 2, C2)[bs]
        s1 = yb.sum(3)
        n = s1.shape[0]
        ysum = s1.reshape(n, HP, 2, HP, C2).sum(2).reshape(n, N1, C2)
        i8, sc = _quant_rows(ysum, pow2=4.0)
        pack(ybuf, bs, i8, sc)

    ex = _executor()
    qs = [slice(b, b + 2) for b in range(0, B, 2)]
    futs = [ex.submit(quant_x, s) for s in qs] + \
           [ex.submit(quant_y, s) for s in qs]
    for f in futs:
        f.result()

    nc = _get_nc(w)
    in_maps = [{"xyN": xybuf[b]} for b in range(B)]
    from concourse.bass_utils import run_bass_kernel_spmd
    res = run_bass_kernel_spmd(nc, in_maps, core_ids=list(range(B)),
                               **_cache.get("run_opts", {}))
    _cache["last_res"] = res

    # dequantize: scale for out row n is packed f32 element n of the tail rows
    outs = np.stack([r["out"] for r in res.results], axis=0)  # (B, N1+25, C1) i8
    i8 = outs[:, :N1, :]
    srow = np.ascontiguousarray(outs[:, N1:, :]).reshape(B, -1) \
             .view(np.float32)[:, :N1] * (1.0 / 127.0)
    return i8 * srow[:, :, None]
